# revision 1
# baseline (speedup 1.0000x reference)
"""Trainium2 Bass kernel: 2-layer GAT (PyG GATConv, heads=1) + per-node actor
MLP + candidate softmax, SPMD across 8 NeuronCores.

Strategy (dst-sharded data parallel):
  - Symmetrized edges + self loops, partitioned by dst across 8 cores,
    grouped into 128-dst blocks. Within a block, edges are sorted by src and
    split into int16-addressable table segments (32768 rows), each padded to
    multiples of 128 edges; the chunk schedule is shared by all cores.
  - Node table per layer: bf16 [z(128) | e_src | 1.0 | pad] rows (512B).
    Per edge, dma_gather pulls the src row (segment-relative int16 idx);
    a second 256B dma_gather pulls e_dst from a shard-local bf16 table.
  - ex = exp(leaky_relu(es+ed)) batched per gather-group; weighted one-hot
    OHw[e,dst] = (iota==d_local)*ex in one DVE op per 128-edge chunk; one PE
    matmul per chunk accumulates numerator AND denominator (table's ones
    column) into the block psum.
  - Epilogue per block: h = elu(num/den + b) (+1 trick), PE transpose,
    projection to next layer's table row + the shard-local ed table.
    Layer boundary: AllGather of the z1 table (bf16).
  - Scores are per-node scalars -> AllGather 400KB -> candidate gather +
    grouped softmax over vm=16, sharded over decisions.
"""

import math
import os
import sys

sys.path.insert(0, "/opt/trn_rl_repo")

import ml_dtypes
import numpy as np

import concourse.bass as bass
import concourse.mybir as mybir
import concourse.tile as tile
from concourse import bacc
from concourse.bass import IndirectOffsetOnAxis
from concourse.bass_utils import run_bass_kernel_spmd

F32 = mybir.dt.float32
I32 = mybir.dt.int32
I16 = mybir.dt.int16
BF16 = mybir.dt.bfloat16
ALU = mybir.AluOpType
ACTF = mybir.ActivationFunctionType
BF = ml_dtypes.bfloat16

NEG_SLOPE = 0.2
P = 128
SEGR = 32768          # table rows per int16-addressable segment
TW = 256              # bf16 table row: z(128) | es | 1.0 | pad  (512B)
GG = 3                # blocks per gather group


# ----------------------------------------------------------------- host prep
def _schedule(edge_index, N, n_cores):
    """Common chunk schedule + per-core index arrays."""
    NSH = N // n_cores
    NBLK = math.ceil(NSH / P)
    NSEG = math.ceil(N / SEGR)
    e0 = edge_index[0].astype(np.int64)
    e1 = edge_index[1].astype(np.int64)
    loops = np.arange(N, dtype=np.int64)
    src = np.concatenate([e0, e1, loops])
    dst = np.concatenate([e1, e0, loops])

    tail = NSH - (NBLK - 1) * P
    n_fake = P - tail if tail < P else 0

    # bucket edges: per core, per block, per segment (src-sorted)
    buckets = [[None] * NSEG for _ in range(NBLK)]
    percore = []
    for c in range(n_cores):
        m = (dst >= c * NSH) & (dst < (c + 1) * NSH)
        s_c, d_c = src[m], dst[m] - c * NSH
        o = np.lexsort((s_c, d_c // P))
        s_c, d_c = s_c[o], d_c[o]
        blk = d_c // P
        bs = np.searchsorted(blk, np.arange(NBLK), side="left")
        be = np.searchsorted(blk, np.arange(NBLK), side="right")
        per_blk = []
        for b in range(NBLK):
            sb_, db_ = s_c[bs[b]:be[b]], d_c[bs[b]:be[b]]
            seg = sb_ >> 15
            segs = []
            for s in range(NSEG):
                sm = seg == s
                segs.append((sb_[sm], db_[sm]))
            per_blk.append(segs)
        percore.append(per_blk)

    # common per (block, seg) chunk counts
    nbs = np.zeros((NBLK, NSEG), dtype=np.int64)
    for b in range(NBLK):
        for s in range(NSEG):
            mx = max(len(percore[c][b][s][0]) for c in range(n_cores))
            if b == NBLK - 1 and s == 0:
                mx += n_fake
            nbs[b, s] = math.ceil(mx / P)
    nb = nbs.sum(axis=1)
    K = int(nb.sum())

    # groups and global column order: per group, seg-major
    groups = []            # (b0, g, kk, [(s, [(b, nchunks)..], colbase)..])
    b0, kk = 0, 0
    while b0 < NBLK:
        g = min(GG, NBLK - b0)
        runs = []
        cb = kk
        for s in range(NSEG):
            blist = [(b, int(nbs[b, s])) for b in range(b0, b0 + g)
                     if nbs[b, s] > 0]
            n = sum(x[1] for x in blist)
            runs.append((s, blist, cb))
            cb += n
        groups.append((b0, g, kk, runs))
        kk = cb
        b0 += g
    assert kk == K

    meta = dict(NSH=NSH, NBLK=NBLK, NSEG=NSEG, nb=[int(x) for x in nb],
                nbs=nbs.tolist(), K=K, groups=groups, n_fake=n_fake, tail=tail)

    out = []
    for c in range(n_cores):
        gsegl = [[] for _ in range(len(groups))]   # seg-rel src list per group
        edl = [[] for _ in range(len(groups))]     # shard-local dst list
        dloc = np.full((P, K), 200.0, dtype=np.float32)
        for gi_, (b0, g, kk, runs) in enumerate(groups):
            for (s, blist, cb) in runs:
                col = cb
                for (b, nch) in blist:
                    sb_, db_ = percore[c][b][s]
                    ns = len(sb_)
                    cap = nch * P
                    sp = np.zeros(cap, dtype=np.int64)
                    ep = np.zeros(cap, dtype=np.int64)
                    dp = np.full(cap, 200.0, dtype=np.float32)
                    sp[:ns] = sb_ - s * SEGR
                    ep[:ns] = db_
                    dp[:ns] = (db_ - b * P).astype(np.float32)
                    if b == NBLK - 1 and s == 0 and n_fake:
                        dp[ns:ns + n_fake] = np.arange(tail, P,
                                                       dtype=np.float32)
                    gsegl[gi_].append((s, sp))
                    edl[gi_].append(ep)
                    dloc[:, col:col + nch] = dp.reshape(nch, P).T
                    col += nch
        # wrapped int16 index tensors, per group contiguous
        gw_parts, goff = [], []
        go = 0
        for gi_ in range(len(groups)):
            segcat = {}
            for (s, sp) in gsegl[gi_]:
                segcat.setdefault(s, []).append(sp)
            slens = []
            for s in sorted(segcat):
                lst = np.concatenate(segcat[s])
                w = lst.reshape(-1, 16).T
                gw_parts.append(np.tile(w, (8, 1)).astype(np.int16))
                slens.append((s, go, len(lst)))
                go += len(lst) // 16
            goff.append(slens)
        out.append(dict(
            gidx16=np.concatenate(gw_parts, axis=1),
            dloc=dloc,
            dloct=np.ascontiguousarray(
                np.broadcast_to(dloc.T[None, :, :], (P, K, P))
            ).astype(BF)))
    meta["goff"] = goff
    return meta, out


def _prep_inputs(inputs, n_cores=8):
    N, IN_DIM = inputs["state_wf"].shape
    HID = inputs["W0"].shape[1]
    VM = 16
    B = inputs["candidate_task_index"].shape[0] // VM
    meta, per_core_e = _schedule(inputs["edge_index"], N, n_cores)
    meta.update(N=N, IN_DIM=IN_DIM, HID=HID, VM=VM, B=B,
                NPAD=math.ceil(N / P) * P)

    f = lambda x: np.asarray(x, dtype=np.float32)
    W0, W1 = f(inputs["W0"]), f(inputs["W1"])
    w0big = np.concatenate(
        [W0, (W0 @ f(inputs["a_src0"]))[:, None],
         (W0 @ f(inputs["a_dst0"]))[:, None]], axis=1)
    w1big = np.concatenate(
        [W1, (W1 @ f(inputs["a_src1"]))[:, None],
         (W1 @ f(inputs["a_dst1"]))[:, None]], axis=1)
    swt = np.zeros((IN_DIM, meta["NPAD"]), dtype=np.float32)
    swt[:, :N] = f(inputs["state_wf"]).T
    NSH, NBLK = meta["NSH"], meta["NBLK"]
    common = dict(
        swt=swt,
        w0big=w0big.astype(np.float32),
        w1big=w1big.astype(np.float32),
        b0t=np.tile(f(inputs["b0"])[None, :], (P, 1)).astype(np.float32),
        b1t=np.tile(f(inputs["b1"])[None, :], (P, 1)).astype(np.float32),
        mw0=f(inputs["mW0"]),
        mw1=f(inputs["mW1"]).reshape(HID, 1),
        mb0=f(inputs["mb0"]).reshape(HID, 1),
        iota=np.tile(np.arange(P, dtype=np.float32)[None, :], (P, 1)),
        iotac=np.arange(P, dtype=np.float32).reshape(P, 1),
        identb=np.eye(P, dtype=BF),
        ident=np.eye(P, dtype=np.float32),
    )
    cand = inputs["candidate_task_index"].astype(np.int64)
    CPC = (B // n_cores) * VM
    CC = CPC // P
    meta["CC"] = CC
    in_maps = []
    for c in range(n_cores):
        m = dict(common)
        m.update(per_core_e[c])
        m["swtsh"] = np.ascontiguousarray(
            np.pad(swt[:, c * NSH:(c + 1) * NSH],
                   ((0, 0), (0, NBLK * P - NSH))))
        m["cidx"] = cand[c * CPC:(c + 1) * CPC].reshape(P, CC).astype(np.int32)
        in_maps.append(m)
    return meta, in_maps


# ------------------------------------------------------------------ builder
def build(meta, n_cores=8):
    NSH, NBLK, NSEG = meta["NSH"], meta["NBLK"], meta["NSEG"]
    nb, K, groups = meta["nb"], meta["K"], meta["groups"]
    N, IN_DIM, HID = meta["N"], meta["IN_DIM"], meta["HID"]
    NPAD, CC, VM = meta["NPAD"], meta["CC"], meta["VM"]
    goff = meta["goff"]
    NTROW = NSEG * SEGR
    IWG = sum(ln // 16 for slens in goff for (_, _, ln) in slens)
    max_gc = max(sum(n for (_, bl, _) in runs for (_, n) in bl)
                 for (_, _, _, runs) in groups)

    STAGE = int(os.environ.get("KERNEL_STAGE", "9"))
    L0P = int(os.environ.get("KERNEL_L0PART", "9"))
    nc = bacc.Bacc("TRN2", target_bir_lowering=False, debug=False,
                   enable_asserts=False, num_devices=n_cores)

    inp = {}
    for name, shape, dt in [
        ("swt", [IN_DIM, NPAD], F32), ("swtsh", [IN_DIM, NBLK * P], F32),
        ("w0big", [IN_DIM, HID + 2], F32), ("w1big", [HID, HID + 2], F32),
        ("b0t", [P, HID], F32), ("b1t", [P, HID], F32),
        ("mw0", [HID, HID], F32), ("mw1", [HID, 1], F32),
        ("mb0", [HID, 1], F32), ("iota", [P, P], F32), ("ident", [P, P], F32),
        ("iotac", [P, 1], F32), ("identb", [P, P], BF16),
        ("dloc", [P, K], F32), ("dloct", [P, K, P], BF16),
        ("gidx16", [P, IWG], I16), ("cidx", [P, CC], I32),
    ]:
        inp[name] = nc.dram_tensor(name, shape, dt, kind="ExternalInput")
    out_t = nc.dram_tensor("out", [P, CC], F32, kind="ExternalOutput")

    z0_tab = nc.dram_tensor("z0tab", [NTROW, TW], BF16, kind="Internal")
    z1_tab = nc.dram_tensor("z1tab", [NTROW, TW], BF16, kind="Internal",
                            addr_space="Shared")
    z1_shard = nc.dram_tensor("z1shard", [NBLK * P, TW], BF16, kind="Internal")
    ed_tab = [nc.dram_tensor(f"ed{l}tab", [NBLK * P, 1], BF16, kind="Internal")
              for l in range(2)]
    sc_shard = nc.dram_tensor("scshard", [NBLK, P, 1], F32, kind="Internal")
    sc_full = nc.dram_tensor("scfull", [N, 1], F32, kind="Internal",
                             addr_space="Shared")

    with tile.TileContext(nc) as tc:
        with (
            tc.tile_pool(name="const", bufs=1) as cpool,
            tc.tile_pool(name="stream", bufs=2) as spool,
            tc.tile_pool(name="idxs", bufs=2) as ipool,
            tc.tile_pool(name="zrows", bufs=2) as zpool,
            tc.tile_pool(name="work", bufs=3) as wpool,
            tc.tile_pool(name="ohwp", bufs=4) as ohwpool,
            tc.tile_pool(name="stage", bufs=2) as stpool,
            tc.tile_pool(name="psacc", bufs=4, space="PSUM") as psacc,
            tc.tile_pool(name="pstp", bufs=2, space="PSUM") as pstp,
            tc.tile_pool(name="psproj", bufs=1, space="PSUM") as psproj,
            tc.tile_pool(name="pssc", bufs=1, space="PSUM") as pssc,
        ):
            sb = {}
            for name in ("w0big", "w1big", "b0t", "b1t", "mw0", "mw1", "mb0",
                         "iota", "ident", "iotac", "identb", "dloc", "cidx"):
                t = inp[name]
                dt = {"cidx": I32, "identb": BF16}.get(name, F32)
                sb[name] = cpool.tile(list(t.shape), dt, tag=name, name=name)
                nc.sync.dma_start(sb[name][:], t[:])
            ones_t = cpool.tile([P, P], BF16, tag="ones_t", name="ones_t")
            nc.vector.memset(ones_t[:], 1.0)

            # ============ phase 0: z0 projection + ed0 table ===============
            SWC = 2048
            NSTG = 8
            for i_sw in range(math.ceil(NPAD / SWC)):
                c0 = i_sw * SWC
                cw = min(SWC, NPAD - c0)
                swt_sb = spool.tile([IN_DIM, 2048], F32, tag="swt", name="swt_t")
                nc.sync.dma_start(swt_sb[:, :cw], inp["swt"][:, c0:c0 + cw])
                for j0 in range(0, cw // P, NSTG):
                    jn = min(NSTG, cw // P - j0)
                    stg = stpool.tile([P, NSTG * TW], BF16, tag="z0st",
                                      name="z0stg")
                    nc.vector.memset(stg[:], 1.0)
                    for j in range(jn):
                        ps = psacc.tile([P, HID + 2], F32, tag="acc",
                                        name="z0ps")
                        nc.tensor.matmul(
                            ps[:], swt_sb[:, (j0 + j) * P:(j0 + j + 1) * P],
                            sb["w0big"][:], start=True, stop=True)
                        if j % 2 == 0:
                            nc.scalar.copy(stg[:, j * TW:j * TW + HID + 1],
                                           ps[:, :HID + 1])
                        else:
                            nc.vector.tensor_copy(
                                stg[:, j * TW:j * TW + HID + 1],
                                ps[:, :HID + 1])
                    r0 = (c0 // P + j0) * P
                    nc.sync.dma_start(
                        z0_tab[r0:r0 + jn * P].rearrange("(j p) c -> p j c",
                                                         p=P),
                        stg[:, :jn * TW].rearrange("p (j c) -> p j c", c=TW))
            # ed0 for own shard (streamed in 16-block chunks)
            EST = 8
            for sb0 in range(0, NBLK, 16):
                sbn = min(16, NBLK - sb0)
                swsh_sb = spool.tile([IN_DIM, 16 * P], F32, tag="swt",
                                     name="swsh")
                nc.sync.dma_start(swsh_sb[:, :sbn * P],
                                  inp["swtsh"][:, sb0 * P:(sb0 + sbn) * P])
                for b0 in range(sb0, sb0 + sbn, EST):
                    bn = min(EST, sb0 + sbn - b0)
                    estg = stpool.tile([P, EST], BF16, tag="edst",
                                       name="edstg")
                    for j in range(bn):
                        b = b0 + j
                        eps = psproj.tile([P, 1], F32, tag="proj", name="edps")
                        nc.tensor.matmul(
                            eps[:], swsh_sb[:, (b - sb0) * P:(b - sb0 + 1) * P],
                            sb["w0big"][:, HID + 1:HID + 2], start=True,
                            stop=True)
                        nc.vector.tensor_copy(estg[:, j:j + 1], eps[:])
                    nc.sync.dma_start(
                        ed_tab[0][b0 * P:(b0 + bn) * P].rearrange(
                            "(j p) c -> p j c", p=P),
                        estg[:, :bn].rearrange("p (j c) -> p j c", c=1))

            # ======================= GAT layers ============================
            for layer in range(2 if STAGE >= 4 else (1 if STAGE >= 2 else 0)):
                btile = sb["b0t"] if layer == 0 else sb["b1t"]
                tab = z0_tab if layer == 0 else z1_tab
                for gi_, (gb0, g, gkk, runs) in enumerate(groups):
                    gc = sum(n for (_, bl, _) in runs for (_, n) in bl)
                    zr = zpool.tile([P, max_gc, TW], BF16, tag="zr", name="zr")
                    dlt = zpool.tile([P, max_gc, P], BF16, tag="dlt",
                                     name="dlt")
                    nc.sync.dma_start(dlt[:, :gc, :],
                                      inp["dloct"][:, gkk:gkk + gc, :])
                    edc = ipool.tile([P, GG], BF16, tag="edc", name="edc")
                    nc.sync.dma_start(
                        edc[:, :g],
                        ed_tab[layer][gb0 * P:(gb0 + g) * P].rearrange(
                            "(j p) c -> p (j c)", p=P))
                    # --- index slices ---
                    for (s, goff_s, ln) in goff[gi_]:
                        gidx = ipool.tile([P, max(ln // 16, 1)], I16,
                                          tag="gidx", name="gidxt")
                        nc.sync.dma_start(
                            gidx[:, :ln // 16],
                            inp["gidx16"][:, goff_s:goff_s + ln // 16])
                        # find col range of this seg run
                        (s_, blist, cb) = runs[s]
                        assert s_ == s
                        nch = ln // P
                        nc.gpsimd.dma_gather(
                            out_ap=zr[:, cb - gkk:cb - gkk + nch, :],
                            in_ap=tab[s * SEGR:(s + 1) * SEGR],
                            idxs_ap=gidx[:, :ln // 16],
                            num_idxs=ln, num_idxs_reg=ln, elem_size=TW,
                            single_packet=False)
                    if L0P < 3:
                        continue
                    # --- chunk matmuls (seg-major order) ---
                    bps = {}
                    first_col = {}
                    last_col = {}
                    for (s, blist, cb) in runs:
                        col = cb
                        for (b, nch) in blist:
                            for k in range(nch):
                                first_col.setdefault(b, col + k)
                                last_col[b] = col + k
                            col += nch
                    if layer == 0:
                        z1stg = stpool.tile([P, GG * TW], BF16, tag="z1st",
                                            name="z1stg")
                        nc.vector.memset(z1stg[:], 1.0)
                        ed1stg = stpool.tile([P, GG], BF16, tag="ed1st",
                                             name="ed1stg")
                    else:
                        scstg = stpool.tile([1, GG * P], F32, tag="scst",
                                            name="scstg")
                    for (s, blist, cb) in runs:
                        col = cb
                        for (b, nch) in blist:
                            if b not in bps:
                                bps[b] = psacc.tile([P, HID + 2], F32,
                                                    tag="acc", name="bps")
                            bloc = b - gb0
                            for k in range(nch):
                                cg = col + k          # global column
                                cl = cg - gkk         # group-local column
                                oht = ohwpool.tile([P, P], BF16, tag="oht",
                                                   name="oht")
                                nc.vector.scalar_tensor_tensor(
                                    out=oht[:],
                                    in0=dlt[:, cl:cl + 1, :].squeeze(),
                                    scalar=sb["iotac"][:],
                                    in1=ones_t[:],
                                    op0=ALU.is_equal, op1=ALU.mult)
                                aps = pstp.tile([P, 1], F32, tag="tp",
                                                name="aps")
                                nc.tensor.matmul(
                                    aps[:], oht[:], edc[:, bloc:bloc + 1],
                                    start=True, stop=False,
                                    skip_group_check=True)
                                nc.tensor.matmul(
                                    aps[:], sb["identb"][:],
                                    zr[:, cl:cl + 1, HID:HID + 1].rearrange(
                                        "p a b -> p (a b)"),
                                    start=False, stop=True,
                                    skip_group_check=True)
                                exc = wpool.tile([P, 1], F32, tag="exc",
                                                 name="exc")
                                nc.scalar.activation(
                                    out=exc[:], in_=aps[:], func=ACTF.Lrelu,
                                    alpha=NEG_SLOPE)
                                nc.scalar.activation(
                                    out=exc[:], in_=exc[:], func=ACTF.Exp)
                                ohw = ohwpool.tile([P, P], BF16, tag="ohw",
                                                   name="ohw")
                                nc.vector.scalar_tensor_tensor(
                                    out=ohw[:], in0=sb["iota"][:],
                                    scalar=sb["dloc"][:, cg:cg + 1],
                                    in1=exc[:].to_broadcast([P, P]),
                                    op0=ALU.is_equal, op1=ALU.mult)
                                nc.tensor.matmul(
                                    bps[b][:], ohw[:],
                                    zr[:, cl:cl + 1, 0:HID + 2].squeeze(),
                                    start=(cg == first_col[b]),
                                    stop=(cg == last_col[b]),
                                    skip_group_check=True)
                            col += nch
                    # --- epilogues (all chunks of the group are done) ---
                    for bi in range(g if L0P >= 4 else 0):
                        b = gb0 + bi
                        pb = bps[b]
                        rc = wpool.tile([P, 1], F32, tag="rc", name="rc")
                        nc.vector.reciprocal(rc[:], pb[:, HID + 1:HID + 2])
                        y = wpool.tile([P, HID], F32, tag="y", name="y")
                        nc.vector.scalar_tensor_tensor(
                            out=y[:], in0=pb[:, :HID], scalar=rc[:],
                            in1=btile[:], op0=ALU.mult, op1=ALU.add)
                        e_t = wpool.tile([P, HID], F32, tag="e_t", name="e_t")
                        r_t = wpool.tile([P, HID], F32, tag="r_t", name="r_t")
                        nc.scalar.activation(out=e_t[:], in_=y[:],
                                             func=ACTF.Exp)
                        nc.scalar.activation(out=r_t[:], in_=y[:],
                                             func=ACTF.Relu)
                        hp1 = wpool.tile([P, HID], F32, tag="hp1", name="hp1")
                        nc.vector.scalar_tensor_tensor(
                            out=hp1[:], in0=e_t[:], scalar=1.0, in1=r_t[:],
                            op0=ALU.min, op1=ALU.add)      # elu(y) + 1
                        tp = pstp.tile([P, P], F32, tag="tp", name="tp")
                        nc.tensor.transpose(tp[:], hp1[:], sb["ident"][:])
                        hT = wpool.tile([P, HID], F32, tag="hT", name="hT")
                        nc.scalar.activation(out=hT[:], in_=tp[:],
                                             func=ACTF.Copy, bias=-1.0)
                        if layer == 0:
                            zps = psproj.tile([P, HID + 2], F32, tag="proj",
                                              name="zps")
                            nc.tensor.matmul(zps[:], hT[:], sb["w1big"][:],
                                             start=True, stop=True,
                                             skip_group_check=True)
                            nc.scalar.copy(
                                z1stg[:, bi * TW:bi * TW + HID + 1],
                                zps[:, :HID + 1])
                            nc.vector.tensor_copy(
                                ed1stg[:, bi:bi + 1],
                                zps[:, HID + 1:HID + 2])
                        else:
                            mps = psproj.tile([P, HID], F32, tag="proj",
                                              name="mps")
                            nc.tensor.matmul(mps[:], sb["mw0"][:], hT[:],
                                             start=True, stop=True,
                                             skip_group_check=True)
                            m1 = wpool.tile([P, HID], F32, tag="m1", name="m1")
                            nc.scalar.activation(out=m1[:], in_=mps[:],
                                                 func=ACTF.Relu,
                                                 bias=sb["mb0"][:])
                            sps = pssc.tile([1, P], F32, tag="sc", name="sps")
                            nc.tensor.matmul(sps[:], sb["mw1"][:], m1[:],
                                             start=True, stop=True,
                                             skip_group_check=True)
                            nc.scalar.copy(scstg[:, bi * P:(bi + 1) * P],
                                           sps[:])
                    if L0P < 4:
                        continue
                    if layer == 0:
                        nc.sync.dma_start(
                            z1_shard[gb0 * P:(gb0 + g) * P].rearrange(
                                "(j p) c -> p j c", p=P),
                            z1stg[:, :g * TW].rearrange("p (j c) -> p j c",
                                                        c=TW))
                        nc.sync.dma_start(
                            ed_tab[1][gb0 * P:(gb0 + g) * P].rearrange(
                                "(j p) c -> p j c", p=P),
                            ed1stg[:, :g].rearrange("p (j c) -> p j c", c=1))
                    else:
                        nc.sync.dma_start(sc_shard[gb0:gb0 + g],
                                          scstg[:, :g * P])
                if layer == 0 and STAGE >= 3:
                    nc.gpsimd.collective_compute(
                        "AllGather", ALU.bypass,
                        replica_groups=[list(range(n_cores))],
                        ins=[z1_shard[:].flatten()[0:NSH * TW].opt()],
                        outs=[z1_tab[0:N].flatten().opt()])

            # ================= scores + candidate softmax ==================
            if STAGE < 5:
                po = wpool.tile([P, CC], F32, tag="pi", name="po")
                nc.vector.memset(po[:], 0.0)
                nc.sync.dma_start(out_t[:], po[:])
            if STAGE >= 5:
              nc.gpsimd.collective_compute(
                  "AllGather", ALU.bypass,
                  replica_groups=[list(range(n_cores))],
                  ins=[sc_shard[:].flatten()[0:NSH].opt()],
                  outs=[sc_full[:].flatten().opt()])
              scg = wpool.tile([P, CC], F32, tag="scg", name="scg")
              for c in range(CC):
                  nc.gpsimd.indirect_dma_start(
                      out=scg[:, c:c + 1], out_offset=None, in_=sc_full[:],
                      in_offset=IndirectOffsetOnAxis(
                          ap=sb["cidx"][:, c:c + 1], axis=0))
              NG = CC // VM
              pex = wpool.tile([P, CC], F32, tag="pex", name="pex")
              nc.scalar.activation(out=pex[:], in_=scg[:], func=ACTF.Exp)
              ssum = wpool.tile([P, NG], F32, tag="ssum", name="ssum")
              nc.vector.tensor_reduce(
                  out=ssum[:], in_=pex[:].rearrange("p (g v) -> p g v", v=VM),
                  axis=mybir.AxisListType.X, op=ALU.add)
              rcg = wpool.tile([P, NG], F32, tag="rcg", name="rcg")
              nc.vector.reciprocal(rcg[:], ssum[:])
              pi = wpool.tile([P, CC], F32, tag="pi", name="pi")
              for g_ in range(NG):
                  nc.vector.tensor_scalar(
                      out=pi[:, g_ * VM:(g_ + 1) * VM],
                      in0=pex[:, g_ * VM:(g_ + 1) * VM],
                      scalar1=rcg[:, g_:g_ + 1], scalar2=0.0,
                      op0=ALU.mult, op1=ALU.add)
              nc.sync.dma_start(out_t[:], pi[:])

            if os.environ.get("KERNEL_DEBUG"):
                dbg_z1 = nc.dram_tensor("dbg_z1", [4, P, TW], F32,
                                        kind="ExternalOutput")
                dbg_sc = nc.dram_tensor("dbg_sc", [NBLK, P, 1], F32,
                                        kind="ExternalOutput")
                tmp = stpool.tile([P, 4 * TW], F32, tag="dbgt", name="dbgt")
                nc.sync.dma_start(
                    tmp[:].rearrange("p (j c) -> p j c", c=TW),
                    z1_shard[0:4 * P].rearrange("(j p) c -> p j c", p=P))
                nc.sync.dma_start(
                    dbg_z1[:].rearrange("j p c -> p j c"),
                    tmp[:].rearrange("p (j c) -> p j c", c=TW))
                tmp2 = stpool.tile([P, NBLK], F32, tag="dbgt2", name="dbgt2")
                nc.sync.dma_start(
                    tmp2[:].rearrange("p (j c) -> p j c", c=1),
                    sc_shard[:].rearrange("j p c -> p j c"))
                nc.sync.dma_start(
                    dbg_sc[:].rearrange("j p c -> p j c"),
                    tmp2[:].rearrange("p (j c) -> p j c", c=1))

    return nc


# ------------------------------------------------------------------- kernel
def kernel(**inputs):
    n_cores = 8
    meta, in_maps = _prep_inputs(inputs, n_cores)
    nc = build(meta, n_cores)
    nc.compile()
    res = run_bass_kernel_spmd(
        nc, in_maps, core_ids=list(range(n_cores)),
        trace=bool(int(os.environ.get("KERNEL_TRACE", "0"))))
    kernel.last_results = res
    kernel.last_meta = meta
    VM = meta["VM"]
    outs = [res.results[c]["out"].reshape(-1, VM) for c in range(n_cores)]
    return np.concatenate(outs, axis=0).astype(np.float32)



# revision 7
# speedup vs baseline: 1.2499x; 1.2499x over previous
"""Trainium2 Bass kernel: 2-layer GAT (PyG GATConv, heads=1) + per-node actor
MLP + candidate softmax, SPMD across 8 NeuronCores.

Strategy (dst-sharded data parallel):
  - Symmetrized edges + self loops, partitioned by dst across 8 cores,
    grouped into 128-dst blocks. Within a block, edges are sorted by src and
    split into int16-addressable table segments (32768 rows), each padded to
    multiples of 128 edges; the chunk schedule is shared by all cores.
  - Node table per layer: bf16 [z(128) | e_src | 1.0 | pad] rows (512B).
    Per edge, dma_gather pulls the src row (segment-relative int16 idx).
  - Host precomputes the transposed one-hot (dst-on-partitions) per chunk,
    streamed bf16; one matmul per chunk gathers e_dst onto edge partitions.
    alpha = es + ed batched per gather-group: DVE add + DVE leaky-relu +
    one Act exp per group (keeps the Act engine inside one act-table set).
  - ohw[e,dst] = (iota==d_local)*ex built in one bf16 DVE op per 128-edge
    chunk; one PE matmul per chunk accumulates numerator AND denominator
    (table's ones column) into the block psum.
  - Epilogue per block: h = elu(num/den + b) (+1 trick), PE transpose,
    projection to next layer's table row + the shard-local ed table.
    Phase 0 and the layer boundary AllGather the bf16 node tables.
  - Scores are per-node scalars -> AllGather 400KB -> candidate gather +
    grouped softmax over vm=16, sharded over decisions.
"""

import math
import os
import sys

sys.path.insert(0, "/opt/trn_rl_repo")

import ml_dtypes
import numpy as np

import concourse.bass as bass
import concourse.mybir as mybir
import concourse.tile as tile
from concourse import bacc
from concourse.bass import IndirectOffsetOnAxis
from concourse.bass_utils import run_bass_kernel_spmd

F32 = mybir.dt.float32
I32 = mybir.dt.int32
I16 = mybir.dt.int16
BF16 = mybir.dt.bfloat16
ALU = mybir.AluOpType
ACTF = mybir.ActivationFunctionType
BF = ml_dtypes.bfloat16

NEG_SLOPE = 0.2
P = 128
SEGR = 32768          # table rows per int16-addressable segment
TW = 256              # bf16 table row: z(128) | es | 1.0 | pad  (512B)
GG = 3                # blocks per gather group


# ----------------------------------------------------------------- host prep
def _schedule(edge_index, N, n_cores):
    """Common chunk schedule + per-core index arrays."""
    NSH = N // n_cores
    NBLK = math.ceil(NSH / P)
    NSEG = math.ceil(N / SEGR)
    e0 = edge_index[0].astype(np.int64)
    e1 = edge_index[1].astype(np.int64)
    loops = np.arange(N, dtype=np.int64)
    src = np.concatenate([e0, e1, loops])
    dst = np.concatenate([e1, e0, loops])

    tail = NSH - (NBLK - 1) * P
    n_fake = P - tail if tail < P else 0

    # bucket edges: per core, per block, per segment (src-sorted)
    buckets = [[None] * NSEG for _ in range(NBLK)]
    percore = []
    for c in range(n_cores):
        m = (dst >= c * NSH) & (dst < (c + 1) * NSH)
        s_c, d_c = src[m], dst[m] - c * NSH
        o = np.lexsort((s_c, d_c // P))
        s_c, d_c = s_c[o], d_c[o]
        blk = d_c // P
        bs = np.searchsorted(blk, np.arange(NBLK), side="left")
        be = np.searchsorted(blk, np.arange(NBLK), side="right")
        per_blk = []
        for b in range(NBLK):
            sb_, db_ = s_c[bs[b]:be[b]], d_c[bs[b]:be[b]]
            seg = sb_ >> 15
            segs = []
            for s in range(NSEG):
                sm = seg == s
                segs.append((sb_[sm], db_[sm]))
            per_blk.append(segs)
        percore.append(per_blk)

    # common per (block, seg) chunk counts
    nbs = np.zeros((NBLK, NSEG), dtype=np.int64)
    for b in range(NBLK):
        for s in range(NSEG):
            mx = max(len(percore[c][b][s][0]) for c in range(n_cores))
            if b == NBLK - 1 and s == 0:
                mx += n_fake
            nbs[b, s] = math.ceil(mx / P)
    nb = nbs.sum(axis=1)
    K = int(nb.sum())

    # groups and global column order: per group, seg-major
    groups = []            # (b0, g, kk, [(s, [(b, nchunks)..], colbase)..])
    b0, kk = 0, 0
    while b0 < NBLK:
        g = min(GG, NBLK - b0)
        runs = []
        cb = kk
        for s in range(NSEG):
            blist = [(b, int(nbs[b, s])) for b in range(b0, b0 + g)
                     if nbs[b, s] > 0]
            n = sum(x[1] for x in blist)
            runs.append((s, blist, cb))
            cb += n
        groups.append((b0, g, kk, runs))
        kk = cb
        b0 += g
    assert kk == K

    meta = dict(NSH=NSH, NBLK=NBLK, NSEG=NSEG, nb=[int(x) for x in nb],
                nbs=nbs.tolist(), K=K, groups=groups, n_fake=n_fake, tail=tail)

    out = []
    for c in range(n_cores):
        gsegl = [[] for _ in range(len(groups))]   # seg-rel src list per group
        dloc = np.full((P, K), 200.0, dtype=np.float32)
        for gi_, (b0, g, kk, runs) in enumerate(groups):
            for (s, blist, cb) in runs:
                col = cb
                for (b, nch) in blist:
                    sb_, db_ = percore[c][b][s]
                    ns = len(sb_)
                    cap = nch * P
                    sp = np.zeros(cap, dtype=np.int64)
                    dp = np.full(cap, 200.0, dtype=np.float32)
                    sp[:ns] = sb_ - s * SEGR
                    dp[:ns] = (db_ - b * P).astype(np.float32)
                    if b == NBLK - 1 and s == 0 and n_fake:
                        dp[ns:ns + n_fake] = np.arange(tail, P,
                                                       dtype=np.float32)
                    gsegl[gi_].append((s, sp))
                    dloc[:, col:col + nch] = dp.reshape(nch, P).T
                    col += nch
        # wrapped int16 index tensors, per group contiguous
        gw_parts, goff = [], []
        go = 0
        for gi_ in range(len(groups)):
            segcat = {}
            for (s, sp) in gsegl[gi_]:
                segcat.setdefault(s, []).append(sp)
            slens = []
            for s in sorted(segcat):
                lst = np.concatenate(segcat[s])
                w = lst.reshape(-1, 16).T
                gw_parts.append(np.tile(w, (8, 1)).astype(np.int16))
                slens.append((s, go, len(lst)))
                go += len(lst) // 16
            goff.append(slens)
        # transposed one-hot per chunk: ohT[p, cg*128 + j] = (dloc[j,cg]==p)
        ohT = (np.arange(P, dtype=np.float32)[:, None, None]
               == dloc.T[None, :, :])
        out.append(dict(
            gidx16=np.concatenate(gw_parts, axis=1),
            dloc=dloc.astype(BF),
            ohT=np.ascontiguousarray(
                ohT.reshape(P, K * P)).astype(BF)))
    meta["goff"] = goff
    return meta, out


def _prep_inputs(inputs, n_cores=8):
    N, IN_DIM = inputs["state_wf"].shape
    HID = inputs["W0"].shape[1]
    VM = 16
    B = inputs["candidate_task_index"].shape[0] // VM
    meta, per_core_e = _schedule(inputs["edge_index"], N, n_cores)
    meta.update(N=N, IN_DIM=IN_DIM, HID=HID, VM=VM, B=B,
                NPAD=math.ceil(N / P) * P)

    f = lambda x: np.asarray(x, dtype=np.float32)
    W0, W1 = f(inputs["W0"]), f(inputs["W1"])
    w0big = np.concatenate(
        [W0, (W0 @ f(inputs["a_src0"]))[:, None],
         (W0 @ f(inputs["a_dst0"]))[:, None]], axis=1)
    w1big = np.concatenate(
        [W1, (W1 @ f(inputs["a_src1"]))[:, None],
         (W1 @ f(inputs["a_dst1"]))[:, None]], axis=1)
    swt = np.zeros((IN_DIM, meta["NPAD"]), dtype=np.float32)
    swt[:, :N] = f(inputs["state_wf"]).T
    NSH, NBLK = meta["NSH"], meta["NBLK"]
    common = dict(
        w0big=w0big.astype(np.float32),
        w1big=w1big.astype(np.float32),
        b0t=np.tile(f(inputs["b0"])[None, :], (P, 1)).astype(np.float32),
        b1t=np.tile(f(inputs["b1"])[None, :], (P, 1)).astype(np.float32),
        mw0=f(inputs["mW0"]),
        mw1=f(inputs["mW1"]).reshape(HID, 1),
        mb0=f(inputs["mb0"]).reshape(HID, 1),
        iota=np.tile(np.arange(P, dtype=np.float32)[None, :],
                     (P, 1)).astype(BF),
        ident=np.eye(P, dtype=np.float32),
    )
    cand = inputs["candidate_task_index"].astype(np.int64)
    CPC = (B // n_cores) * VM
    CC = CPC // P
    meta["CC"] = CC
    in_maps = []
    for c in range(n_cores):
        m = dict(common)
        m.update(per_core_e[c])
        m["swtsh"] = np.ascontiguousarray(
            np.pad(swt[:, c * NSH:(c + 1) * NSH],
                   ((0, 0), (0, NBLK * P - NSH))))
        m["cidx"] = cand[c * CPC:(c + 1) * CPC].reshape(P, CC).astype(np.int32)
        in_maps.append(m)
    return meta, in_maps


# ------------------------------------------------------------------ builder
def build(meta, n_cores=8):
    NSH, NBLK, NSEG = meta["NSH"], meta["NBLK"], meta["NSEG"]
    nb, K, groups = meta["nb"], meta["K"], meta["groups"]
    N, IN_DIM, HID = meta["N"], meta["IN_DIM"], meta["HID"]
    NPAD, CC, VM = meta["NPAD"], meta["CC"], meta["VM"]
    goff = meta["goff"]
    NTROW = NSEG * SEGR
    IWG = sum(ln // 16 for slens in goff for (_, _, ln) in slens)
    max_gc = max(sum(n for (_, bl, _) in runs for (_, n) in bl)
                 for (_, _, _, runs) in groups)

    STAGE = int(os.environ.get("KERNEL_STAGE", "9"))
    SP = bool(int(os.environ.get("KERNEL_SP", "0")))
    nc = bacc.Bacc("TRN2", target_bir_lowering=False, debug=False,
                   enable_asserts=False, num_devices=n_cores)

    inp = {}
    for name, shape, dt in [
        ("swtsh", [IN_DIM, NBLK * P], F32),
        ("w0big", [IN_DIM, HID + 2], F32), ("w1big", [HID, HID + 2], F32),
        ("b0t", [P, HID], F32), ("b1t", [P, HID], F32),
        ("mw0", [HID, HID], F32), ("mw1", [HID, 1], F32),
        ("mb0", [HID, 1], F32), ("iota", [P, P], BF16),
        ("ident", [P, P], F32),
        ("dloc", [P, K], BF16), ("ohT", [P, K * P], BF16),
        ("gidx16", [P, IWG], I16), ("cidx", [P, CC], I32),
    ]:
        inp[name] = nc.dram_tensor(name, shape, dt, kind="ExternalInput")
    out_t = nc.dram_tensor("out", [P, CC], F32, kind="ExternalOutput")

    z0_shard = nc.dram_tensor("z0shard", [NBLK * P, TW], BF16, kind="Internal")
    z0_tab = nc.dram_tensor("z0tab", [NTROW, TW], BF16, kind="Internal",
                            addr_space="Shared")
    z1_tab = nc.dram_tensor("z1tab", [NTROW, TW], BF16, kind="Internal",
                            addr_space="Shared")
    z1_shard = nc.dram_tensor("z1shard", [NBLK * P, TW], BF16, kind="Internal")
    ed_tab = [nc.dram_tensor(f"ed{l}tab", [NBLK * P, 1], BF16, kind="Internal")
              for l in range(2)]
    sc_shard = nc.dram_tensor("scshard", [NBLK, P, 1], F32, kind="Internal")
    sc_full = nc.dram_tensor("scfull", [N, 1], F32, kind="Internal",
                             addr_space="Shared")

    with tile.TileContext(nc) as tc:
        with (
            tc.tile_pool(name="const", bufs=1) as cpool,
            tc.tile_pool(name="stream", bufs=2) as spool,
            tc.tile_pool(name="idxs", bufs=2) as ipool,
            tc.tile_pool(name="zrows", bufs=2) as zpool,
            tc.tile_pool(name="ohts", bufs=2) as opool,
            tc.tile_pool(name="work", bufs=3) as wpool,
            tc.tile_pool(name="ohwp", bufs=4) as ohwpool,
            tc.tile_pool(name="stage", bufs=2) as stpool,
            tc.tile_pool(name="psacc", bufs=3, space="PSUM") as psacc,
            tc.tile_pool(name="psaps", bufs=2, space="PSUM") as psaps,
            tc.tile_pool(name="pstp", bufs=1, space="PSUM") as pstp,
            tc.tile_pool(name="psproj", bufs=1, space="PSUM") as psproj,
            tc.tile_pool(name="pssc", bufs=1, space="PSUM") as pssc,
        ):
            sb = {}
            for name in ("w0big", "w1big", "b0t", "b1t", "mw0", "mw1", "mb0",
                         "iota", "ident", "dloc", "cidx"):
                t = inp[name]
                dt = {"cidx": I32, "iota": BF16, "dloc": BF16}.get(name, F32)
                sb[name] = cpool.tile(list(t.shape), dt, tag=name, name=name)
                nc.sync.dma_start(sb[name][:], t[:])

            # ===== phase 0: shard z0 projection + ed0 table + AllGather ====
            NSTG = 8
            for sb0 in range(0, NBLK, 16):
                sbn = min(16, NBLK - sb0)
                swsh_sb = spool.tile([IN_DIM, 16 * P], F32, tag="swt",
                                     name="swsh")
                nc.sync.dma_start(swsh_sb[:, :sbn * P],
                                  inp["swtsh"][:, sb0 * P:(sb0 + sbn) * P])
                for j0 in range(0, sbn, NSTG):
                    jn = min(NSTG, sbn - j0)
                    stg = stpool.tile([P, NSTG * TW], BF16, tag="z0st",
                                      name="z0stg")
                    nc.vector.memset(stg[:], 1.0)
                    estg = stpool.tile([P, NSTG], BF16, tag="edst",
                                       name="edstg")
                    for j in range(jn):
                        bl = j0 + j
                        ps = psacc.tile([P, HID + 2], F32, tag="acc",
                                        name="z0ps")
                        nc.tensor.matmul(
                            ps[:], swsh_sb[:, bl * P:(bl + 1) * P],
                            sb["w0big"][:], start=True, stop=True)
                        if j % 2 == 0:
                            nc.scalar.copy(stg[:, j * TW:j * TW + HID + 1],
                                           ps[:, :HID + 1])
                        else:
                            nc.vector.tensor_copy(
                                stg[:, j * TW:j * TW + HID + 1],
                                ps[:, :HID + 1])
                        nc.vector.tensor_copy(estg[:, j:j + 1],
                                              ps[:, HID + 1:HID + 2])
                    r0 = (sb0 + j0) * P
                    nc.sync.dma_start(
                        z0_shard[r0:r0 + jn * P].rearrange("(j p) c -> p j c",
                                                           p=P),
                        stg[:, :jn * TW].rearrange("p (j c) -> p j c", c=TW))
                    nc.sync.dma_start(
                        ed_tab[0][r0:r0 + jn * P].rearrange(
                            "(j p) c -> p j c", p=P),
                        estg[:, :jn].rearrange("p (j c) -> p j c", c=1))
            nc.gpsimd.collective_compute(
                "AllGather", ALU.bypass,
                replica_groups=[list(range(n_cores))],
                ins=[z0_shard[:].flatten()[0:NSH * TW].opt()],
                outs=[z0_tab[0:N].flatten().opt()])

            # ======================= GAT layers ============================
            for layer in range(2 if STAGE >= 4 else (1 if STAGE >= 2 else 0)):
                btile = sb["b0t"] if layer == 0 else sb["b1t"]
                tab = z0_tab if layer == 0 else z1_tab
                for gi_, (gb0, g, gkk, runs) in enumerate(groups):
                    gc = sum(n for (_, bl, _) in runs for (_, n) in bl)
                    zr = zpool.tile([P, max_gc, TW], BF16, tag="zr", name="zr")
                    oht = opool.tile([P, max_gc * P], BF16, tag="oht",
                                     name="oht")
                    nc.sync.dma_start(oht[:, :gc * P],
                                      inp["ohT"][:, gkk * P:(gkk + gc) * P])
                    edc = ipool.tile([P, GG], BF16, tag="edc", name="edc")
                    nc.sync.dma_start(
                        edc[:, :g],
                        ed_tab[layer][gb0 * P:(gb0 + g) * P].rearrange(
                            "(j p) c -> p (j c)", p=P))
                    # --- gathers (per index segment) ---
                    for (s, goff_s, ln) in goff[gi_]:
                        gidx = ipool.tile([P, max(ln // 16, 1)], I16,
                                          tag="gidx", name="gidxt")
                        nc.sync.dma_start(
                            gidx[:, :ln // 16],
                            inp["gidx16"][:, goff_s:goff_s + ln // 16])
                        (s_, blist, cb) = runs[s]
                        assert s_ == s
                        nch = ln // P
                        nc.gpsimd.dma_gather(
                            out_ap=zr[:, cb - gkk:cb - gkk + nch, :],
                            in_ap=tab[s * SEGR:(s + 1) * SEGR],
                            idxs_ap=gidx[:, :ln // 16],
                            num_idxs=ln, num_idxs_reg=ln, elem_size=TW,
                            single_packet=SP)
                    # --- pass 1: ed per edge via one-hot matmuls ---
                    first_col = {}
                    last_col = {}
                    for (s, blist, cb) in runs:
                        col = cb
                        for (b, nch) in blist:
                            for k in range(nch):
                                first_col.setdefault(b, col + k)
                                last_col[b] = col + k
                            col += nch
                    aps = psaps.tile([P, max_gc], F32, tag="aps", name="aps")
                    for (s, blist, cb) in runs:
                        col = cb
                        for (b, nch) in blist:
                            bloc = b - gb0
                            for k in range(nch):
                                cl = col + k - gkk
                                nc.tensor.matmul(
                                    aps[:, cl:cl + 1],
                                    oht[:, cl * P:(cl + 1) * P],
                                    edc[:, bloc:bloc + 1],
                                    start=True, stop=True,
                                    skip_group_check=True)
                            col += nch
                    # --- alpha = es + ed, lrelu, exp (batched per group) ---
                    tse = wpool.tile([P, max_gc], F32, tag="tse", name="tse")
                    nc.vector.tensor_tensor(
                        out=tse[:, :gc], in0=aps[:, :gc],
                        in1=zr[:, :gc, HID:HID + 1].rearrange(
                            "p a b -> p (a b)"),
                        op=ALU.add)
                    lr = wpool.tile([P, max_gc], F32, tag="lr", name="lr")
                    nc.vector.scalar_tensor_tensor(
                        out=lr[:, :gc], in0=tse[:, :gc], scalar=NEG_SLOPE,
                        in1=tse[:, :gc], op0=ALU.mult, op1=ALU.max)
                    exc = wpool.tile([P, max_gc], BF16, tag="exc", name="exc")
                    nc.scalar.activation(out=exc[:, :gc], in_=lr[:, :gc],
                                         func=ACTF.Exp)
                    # --- pass 2: weighted one-hot + scatter matmuls ---
                    bps = {}
                    if layer == 0:
                        z1stg = stpool.tile([P, GG * TW], BF16, tag="z1st",
                                            name="z1stg")
                        nc.vector.memset(z1stg[:], 1.0)
                        ed1stg = stpool.tile([P, GG], BF16, tag="ed1st",
                                             name="ed1stg")
                    else:
                        scstg = stpool.tile([1, GG * P], F32, tag="scst",
                                            name="scstg")
                    for (s, blist, cb) in runs:
                        col = cb
                        for (b, nch) in blist:
                            if b not in bps:
                                bps[b] = psacc.tile([P, HID + 2], F32,
                                                    tag="acc", name="bps")
                            for k in range(nch):
                                cg = col + k          # global column
                                cl = cg - gkk         # group-local column
                                ohw = ohwpool.tile([P, P], BF16, tag="ohw",
                                                   name="ohw")
                                nc.vector.scalar_tensor_tensor(
                                    out=ohw[:], in0=sb["iota"][:],
                                    scalar=sb["dloc"][:, cg:cg + 1],
                                    in1=exc[:, cl:cl + 1].to_broadcast(
                                        [P, P]),
                                    op0=ALU.is_equal, op1=ALU.mult)
                                nc.tensor.matmul(
                                    bps[b][:], ohw[:],
                                    zr[:, cl:cl + 1, 0:HID + 2].squeeze(),
                                    start=(cg == first_col[b]),
                                    stop=(cg == last_col[b]),
                                    skip_group_check=True)
                            col += nch
                    # --- epilogues (all chunks of the group are done) ---
                    for bi in range(g):
                        b = gb0 + bi
                        pb = bps[b]
                        rc = wpool.tile([P, 1], F32, tag="rc", name="rc")
                        nc.vector.reciprocal(rc[:], pb[:, HID + 1:HID + 2])
                        y = wpool.tile([P, HID], F32, tag="y", name="y")
                        nc.vector.scalar_tensor_tensor(
                            out=y[:], in0=pb[:, :HID], scalar=rc[:],
                            in1=btile[:], op0=ALU.mult, op1=ALU.add)
                        e_t = wpool.tile([P, HID], F32, tag="e_t", name="e_t")
                        r_t = wpool.tile([P, HID], F32, tag="r_t", name="r_t")
                        nc.scalar.activation(out=e_t[:], in_=y[:],
                                             func=ACTF.Exp)
                        nc.scalar.activation(out=r_t[:], in_=y[:],
                                             func=ACTF.Relu)
                        hp1 = wpool.tile([P, HID], F32, tag="hp1", name="hp1")
                        nc.vector.scalar_tensor_tensor(
                            out=hp1[:], in0=e_t[:], scalar=1.0, in1=r_t[:],
                            op0=ALU.min, op1=ALU.add)      # elu(y) + 1
                        tp = pstp.tile([P, P], F32, tag="tp", name="tp")
                        nc.tensor.transpose(tp[:], hp1[:], sb["ident"][:])
                        hT = wpool.tile([P, HID], F32, tag="hT", name="hT")
                        nc.scalar.activation(out=hT[:], in_=tp[:],
                                             func=ACTF.Copy, bias=-1.0)
                        if layer == 0:
                            zps = psproj.tile([P, HID + 2], F32, tag="proj",
                                              name="zps")
                            nc.tensor.matmul(zps[:], hT[:], sb["w1big"][:],
                                             start=True, stop=True,
                                             skip_group_check=True)
                            nc.scalar.copy(
                                z1stg[:, bi * TW:bi * TW + HID + 1],
                                zps[:, :HID + 1])
                            nc.vector.tensor_copy(
                                ed1stg[:, bi:bi + 1],
                                zps[:, HID + 1:HID + 2])
                        else:
                            mps = psproj.tile([P, HID], F32, tag="proj",
                                              name="mps")
                            nc.tensor.matmul(mps[:], sb["mw0"][:], hT[:],
                                             start=True, stop=True,
                                             skip_group_check=True)
                            m1 = wpool.tile([P, HID], F32, tag="m1", name="m1")
                            nc.scalar.activation(out=m1[:], in_=mps[:],
                                                 func=ACTF.Relu,
                                                 bias=sb["mb0"][:])
                            sps = pssc.tile([1, P], F32, tag="sc",
                                            name="sps")
                            nc.tensor.matmul(sps[:], sb["mw1"][:], m1[:],
                                             start=True, stop=True,
                                             skip_group_check=True)
                            nc.scalar.copy(scstg[:, bi * P:(bi + 1) * P],
                                           sps[:])
                    if layer == 0:
                        nc.sync.dma_start(
                            z1_shard[gb0 * P:(gb0 + g) * P].rearrange(
                                "(j p) c -> p j c", p=P),
                            z1stg[:, :g * TW].rearrange("p (j c) -> p j c",
                                                        c=TW))
                        nc.sync.dma_start(
                            ed_tab[1][gb0 * P:(gb0 + g) * P].rearrange(
                                "(j p) c -> p j c", p=P),
                            ed1stg[:, :g].rearrange("p (j c) -> p j c", c=1))
                    else:
                        nc.sync.dma_start(sc_shard[gb0:gb0 + g],
                                          scstg[:, :g * P])
                if layer == 0 and STAGE >= 3:
                    nc.gpsimd.collective_compute(
                        "AllGather", ALU.bypass,
                        replica_groups=[list(range(n_cores))],
                        ins=[z1_shard[:].flatten()[0:NSH * TW].opt()],
                        outs=[z1_tab[0:N].flatten().opt()])

            # ================= scores + candidate softmax ==================
            if STAGE < 5:
                po = wpool.tile([P, CC], F32, tag="pi", name="po")
                nc.vector.memset(po[:], 0.0)
                nc.sync.dma_start(out_t[:], po[:])
            if STAGE >= 5:
              nc.gpsimd.collective_compute(
                  "AllGather", ALU.bypass,
                  replica_groups=[list(range(n_cores))],
                  ins=[sc_shard[:].flatten()[0:NSH].opt()],
                  outs=[sc_full[:].flatten().opt()])
              scg = wpool.tile([P, CC], F32, tag="scg", name="scg")
              for c in range(CC):
                  nc.gpsimd.indirect_dma_start(
                      out=scg[:, c:c + 1], out_offset=None, in_=sc_full[:],
                      in_offset=IndirectOffsetOnAxis(
                          ap=sb["cidx"][:, c:c + 1], axis=0))
              NG = CC // VM
              pex = wpool.tile([P, CC], F32, tag="pex", name="pex")
              nc.scalar.activation(out=pex[:], in_=scg[:], func=ACTF.Exp)
              ssum = wpool.tile([P, NG], F32, tag="ssum", name="ssum")
              nc.vector.tensor_reduce(
                  out=ssum[:], in_=pex[:].rearrange("p (g v) -> p g v", v=VM),
                  axis=mybir.AxisListType.X, op=ALU.add)
              rcg = wpool.tile([P, NG], F32, tag="rcg", name="rcg")
              nc.vector.reciprocal(rcg[:], ssum[:])
              pi = wpool.tile([P, CC], F32, tag="pi", name="pi")
              for g_ in range(NG):
                  nc.vector.tensor_scalar(
                      out=pi[:, g_ * VM:(g_ + 1) * VM],
                      in0=pex[:, g_ * VM:(g_ + 1) * VM],
                      scalar1=rcg[:, g_:g_ + 1], scalar2=0.0,
                      op0=ALU.mult, op1=ALU.add)
              nc.sync.dma_start(out_t[:], pi[:])

    return nc


# ------------------------------------------------------------------- kernel
def kernel(**inputs):
    n_cores = 8
    meta, in_maps = _prep_inputs(inputs, n_cores)
    nc = build(meta, n_cores)
    nc.compile()
    res = run_bass_kernel_spmd(
        nc, in_maps, core_ids=list(range(n_cores)),
        trace=bool(int(os.environ.get("KERNEL_TRACE", "0"))))
    kernel.last_results = res
    kernel.last_meta = meta
    VM = meta["VM"]
    outs = [res.results[c]["out"].reshape(-1, VM) for c in range(n_cores)]
    return np.concatenate(outs, axis=0).astype(np.float32)


# revision 8
# speedup vs baseline: 1.7505x; 1.4006x over previous
"""Trainium2 Bass kernel: 2-layer GAT (PyG GATConv, heads=1) + per-node actor
MLP + candidate softmax, SPMD across 8 NeuronCores.

Strategy (dst-sharded data parallel):
  - Symmetrized edges (self loops handled separately), partitioned by dst
    across 8 cores, grouped into 128-dst blocks, GG blocks per gather group.
    Per (group, segment) the edges are packed block-major into one padded
    run of 128-edge chunks; a chunk may span adjacent blocks, handled by
    per-(chunk, block) matmul instances.
  - Node table per layer: bf16 [z(128) | e_src | 1.0 | pad] rows (512B).
    Per edge, dma_gather pulls the src row (segment-relative int16 idx).
  - Host precomputes the transposed one-hot (dst-on-partitions) per
    instance, streamed bf16; one matmul per instance gathers e_dst onto
    edge partitions. alpha = es + ed batched per group: DVE add + DVE
    leaky-relu + one Act exp (keeps Act inside one act-table set).
  - ohw[e,dst] = (iota==d_local)*ex built in one bf16 DVE op per instance;
    one PE matmul per instance accumulates numerator AND denominator
    (table's ones column) into the block psum. Self-loop contributions are
    added per block via a diagonal matmul from the local z-shard staging.
  - Epilogue per block: h = elu(num/den + b) (+1 trick), PE transpose,
    projection to next layer's table row + the shard-local ed table.
    Phase 0 and the layer boundary AllGather the bf16 node tables.
  - Scores are per-node scalars -> AllGather 400KB -> candidate gather +
    grouped softmax over vm=16, sharded over decisions.
"""

import math
import os
import sys

sys.path.insert(0, "/opt/trn_rl_repo")

import ml_dtypes
import numpy as np

import concourse.bass as bass
import concourse.mybir as mybir
import concourse.tile as tile
from concourse import bacc
from concourse.bass import IndirectOffsetOnAxis
from concourse.bass_utils import run_bass_kernel_spmd

F32 = mybir.dt.float32
I32 = mybir.dt.int32
I16 = mybir.dt.int16
BF16 = mybir.dt.bfloat16
ALU = mybir.AluOpType
ACTF = mybir.ActivationFunctionType
BF = ml_dtypes.bfloat16

NEG_SLOPE = 0.2
P = 128
SEGR = 32768          # table rows per int16-addressable segment
TW = 256              # bf16 table row: z(128) | es | 1.0 | pad  (512B)
GG = 3                # blocks per gather group


# ----------------------------------------------------------------- host prep
def _schedule(edge_index, N, n_cores):
    """Common chunk/instance schedule + per-core index arrays."""
    NSH = N // n_cores
    NBLK = math.ceil(NSH / P)
    NSEG = math.ceil(N / SEGR)
    e0 = edge_index[0].astype(np.int64)
    e1 = edge_index[1].astype(np.int64)
    src = np.concatenate([e0, e1])
    dst = np.concatenate([e1, e0])

    # bucket edges: per core, per block, per segment (src-sorted)
    percore = []
    for c in range(n_cores):
        m = (dst >= c * NSH) & (dst < (c + 1) * NSH)
        s_c, d_c = src[m], dst[m] - c * NSH
        o = np.lexsort((s_c, d_c // P))
        s_c, d_c = s_c[o], d_c[o]
        blk = d_c // P
        bs = np.searchsorted(blk, np.arange(NBLK), side="left")
        be = np.searchsorted(blk, np.arange(NBLK), side="right")
        per_blk = []
        for b in range(NBLK):
            sb_, db_ = s_c[bs[b]:be[b]], d_c[bs[b]:be[b]]
            seg = sb_ >> 15
            segs = []
            for s in range(NSEG):
                sm = seg == s
                segs.append((sb_[sm], db_[sm]))
            per_blk.append(segs)
        percore.append(per_blk)

    # common per (group, seg) padded run lengths (shared by all cores) and
    # per (group, seg, block) edge counts per core to derive instance spans
    ngrp = math.ceil(NBLK / GG)
    run_len = np.zeros((ngrp, NSEG), dtype=np.int64)   # padded (x128)
    for gi in range(ngrp):
        b0 = gi * GG
        g = min(GG, NBLK - b0)
        for s in range(NSEG):
            mx = 0
            for c in range(n_cores):
                tot = sum(len(percore[c][b0 + bb][s][0]) for bb in range(g))
                mx = max(mx, tot)
            run_len[gi, s] = math.ceil(mx / P) * P if mx else 0

    # groups meta: per group, per seg: chunk col base; chunk count
    groups = []          # (b0, g, segs=[(s, cb_chunk, nch)], gc)
    kk = 0
    for gi in range(ngrp):
        b0 = gi * GG
        g = min(GG, NBLK - b0)
        segs = []
        for s in range(NSEG):
            nch = int(run_len[gi, s]) // P
            segs.append((s, kk, nch))
            kk += nch
        gc = sum(x[2] for x in segs)
        groups.append((b0, g, segs))
    K = kk

    # per-core: index streams, per-instance dloc and instance schedule.
    # The instance schedule (which blocks each chunk touches) must be
    # IDENTICAL across cores (SPMD single program): merge spans over cores.
    # For each (group, seg, chunk) the set of possibly-touching blocks is
    # derived from per-core block spans; union over cores.
    inst_sets = [dict() for _ in range(ngrp)]   # (s, chunk) -> set(blocks)
    percore_edges = []
    for c in range(n_cores):
        ge = []
        for gi, (b0, g, segs) in enumerate(groups):
            for (s, cb, nch) in segs:
                if nch == 0:
                    continue
                cap = nch * P
                sp = np.zeros(cap, dtype=np.int64)
                dp = np.full(cap, 200.0, dtype=np.float32)
                bl = np.full(cap, -1, dtype=np.int64)
                off = 0
                for bb in range(g):
                    sb_, db_ = percore[c][b0 + bb][s]
                    ns = len(sb_)
                    sp[off:off + ns] = sb_ - s * SEGR
                    dp[off:off + ns] = (db_ - (b0 + bb) * P)
                    bl[off:off + ns] = bb
                    off += ns
                for k in range(nch):
                    touched = set(bl[k * P:(k + 1) * P].tolist()) - {-1}
                    key = (s, cb + k)
                    inst_sets[gi].setdefault(key, set()).update(touched)
                ge.append((gi, s, cb, nch, sp, dp, bl))
        percore_edges.append(ge)

    # canonical instance order per group: seg-major, chunk-major, block asc
    inst_meta = []       # per group: list of (s, chunk_col, bb)
    for gi, (b0, g, segs) in enumerate(groups):
        il = []
        for (s, cb, nch) in segs:
            for k in range(nch):
                bbs = sorted(inst_sets[gi].get((s, cb + k), set()))
                if not bbs:
                    bbs = [g - 1]          # dummy all-pad chunk
                for bb in bbs:
                    il.append((s, cb + k, bb))
        inst_meta.append(il)
    I = sum(len(il) for il in inst_meta)

    # build per-group kernel schedules
    gsched = []
    icol = 0
    for gi, (b0, g, segs) in enumerate(groups):
        il = inst_meta[gi]
        # ed-matmul first/last per chunk; acc last per block
        by_chunk = {}
        by_blk = {}
        insts = []
        for j, (s, cl, bb) in enumerate(il):
            by_chunk.setdefault(cl, []).append(j)
            by_blk.setdefault(bb, []).append(j)
        for j, (s, cl, bb) in enumerate(il):
            insts.append(dict(
                cl=cl, bb=bb, icol=icol + j,
                ed_first=(j == by_chunk[cl][0]),
                ed_last=(j == by_chunk[cl][-1]),
                acc_last=(j == by_blk[bb][-1])))
        gc = sum(x[2] for x in segs)
        gsched.append(dict(b0=b0, g=g, segs=segs, gc=gc, insts=insts,
                           icol0=icol, ni=len(il)))
        icol += len(il)
    assert icol == I

    meta = dict(NSH=NSH, NBLK=NBLK, NSEG=NSEG, K=K, I=I, groups=gsched)

    # per-core tensors
    out = []
    for c in range(n_cores):
        dlocI = np.full((P, I), 200.0, dtype=np.float32)
        gw_parts, goff = [], []
        go = 0
        chunk_dp = {}
        for (gi, s, cb, nch, sp, dp, bl) in percore_edges[c]:
            for k in range(nch):
                chunk_dp[(gi, s, cb + k)] = (dp[k * P:(k + 1) * P],
                                             bl[k * P:(k + 1) * P])
        for gi, (b0, g, segs) in enumerate(groups):
            slens = []
            for (gi2, s, cb, nch, sp, dp, bl) in percore_edges[c]:
                if gi2 != gi:
                    continue
                w = sp.reshape(-1, 16).T
                gw_parts.append(np.tile(w, (8, 1)).astype(np.int16))
                slens.append((s, go, len(sp)))
                go += len(sp) // 16
            goff.append(slens)
            for inst in gsched[gi]["insts"]:
                cl, bb, ic = inst["cl"], inst["bb"], inst["icol"]
                # find seg of this chunk
                for (s, cb, nch) in segs:
                    if cb <= cl < cb + nch:
                        break
                dpk, blk_ = chunk_dp.get((gi, s, cl), (None, None))
                if dpk is None:
                    continue
                v = np.where(blk_ == bb, dpk, 200.0)
                dlocI[:, ic] = v
        ohT = (np.arange(P, dtype=np.float32)[:, None, None]
               == dlocI.T[None, :, :])
        out.append(dict(
            gidx16=np.concatenate(gw_parts, axis=1),
            dloc=dlocI.astype(BF),
            ohT=np.ascontiguousarray(ohT.reshape(P, I * P)).astype(BF)))
    meta["goff"] = goff
    return meta, out


def _prep_inputs(inputs, n_cores=8):
    N, IN_DIM = inputs["state_wf"].shape
    HID = inputs["W0"].shape[1]
    VM = 16
    B = inputs["candidate_task_index"].shape[0] // VM
    meta, per_core_e = _schedule(inputs["edge_index"], N, n_cores)
    meta.update(N=N, IN_DIM=IN_DIM, HID=HID, VM=VM, B=B,
                NPAD=math.ceil(N / P) * P)

    f = lambda x: np.asarray(x, dtype=np.float32)
    W0, W1 = f(inputs["W0"]), f(inputs["W1"])
    w0big = np.concatenate(
        [W0, (W0 @ f(inputs["a_src0"]))[:, None],
         (W0 @ f(inputs["a_dst0"]))[:, None]], axis=1)
    w1big = np.concatenate(
        [W1, (W1 @ f(inputs["a_src1"]))[:, None],
         (W1 @ f(inputs["a_dst1"]))[:, None]], axis=1)
    swt = np.zeros((IN_DIM, meta["NPAD"]), dtype=np.float32)
    swt[:, :N] = f(inputs["state_wf"]).T
    NSH, NBLK = meta["NSH"], meta["NBLK"]
    common = dict(
        w0big=w0big.astype(np.float32),
        w1big=w1big.astype(np.float32),
        b0t=np.tile(f(inputs["b0"])[None, :], (P, 1)).astype(np.float32),
        b1t=np.tile(f(inputs["b1"])[None, :], (P, 1)).astype(np.float32),
        mw0=f(inputs["mW0"]),
        mw1=f(inputs["mW1"]).reshape(HID, 1),
        mb0=f(inputs["mb0"]).reshape(HID, 1),
        iota=np.tile(np.arange(P, dtype=np.float32)[None, :],
                     (P, 1)).astype(BF),
        iotac=np.arange(P, dtype=np.float32).reshape(P, 1).astype(BF),
        ident=np.eye(P, dtype=np.float32),
    )
    cand = inputs["candidate_task_index"].astype(np.int64)
    CPC = (B // n_cores) * VM
    CC = CPC // P
    meta["CC"] = CC
    in_maps = []
    for c in range(n_cores):
        m = dict(common)
        m.update(per_core_e[c])
        m["swtsh"] = np.ascontiguousarray(
            np.pad(swt[:, c * NSH:(c + 1) * NSH],
                   ((0, 0), (0, NBLK * P - NSH))))
        m["cidx"] = cand[c * CPC:(c + 1) * CPC].reshape(P, CC).astype(np.int32)
        in_maps.append(m)
    return meta, in_maps


# ------------------------------------------------------------------ builder
def build(meta, n_cores=8):
    NSH, NBLK, NSEG = meta["NSH"], meta["NBLK"], meta["NSEG"]
    K, I, groups = meta["K"], meta["I"], meta["groups"]
    N, IN_DIM, HID = meta["N"], meta["IN_DIM"], meta["HID"]
    NPAD, CC, VM = meta["NPAD"], meta["CC"], meta["VM"]
    goff = meta["goff"]
    NTROW = NSEG * SEGR
    IWG = sum(ln // 16 for slens in goff for (_, _, ln) in slens)
    max_gc = max(gs["gc"] for gs in groups)
    max_ni = max(gs["ni"] for gs in groups)

    SP = bool(int(os.environ.get("KERNEL_SP", "0")))
    nc = bacc.Bacc("TRN2", target_bir_lowering=False, debug=False,
                   enable_asserts=False, num_devices=n_cores)

    inp = {}
    for name, shape, dt in [
        ("swtsh", [IN_DIM, NBLK * P], F32),
        ("w0big", [IN_DIM, HID + 2], F32), ("w1big", [HID, HID + 2], F32),
        ("b0t", [P, HID], F32), ("b1t", [P, HID], F32),
        ("mw0", [HID, HID], F32), ("mw1", [HID, 1], F32),
        ("mb0", [HID, 1], F32), ("iota", [P, P], BF16),
        ("iotac", [P, 1], BF16), ("ident", [P, P], F32),
        ("dloc", [P, I], BF16), ("ohT", [P, I * P], BF16),
        ("gidx16", [P, IWG], I16), ("cidx", [P, CC], I32),
    ]:
        inp[name] = nc.dram_tensor(name, shape, dt, kind="ExternalInput")
    out_t = nc.dram_tensor("out", [P, CC], F32, kind="ExternalOutput")

    z0_shard = nc.dram_tensor("z0shard", [NBLK * P, TW], BF16, kind="Internal")
    z0_tab = nc.dram_tensor("z0tab", [NTROW, TW], BF16, kind="Internal",
                            addr_space="Shared")
    z1_tab = nc.dram_tensor("z1tab", [NTROW, TW], BF16, kind="Internal",
                            addr_space="Shared")
    z1_shard = nc.dram_tensor("z1shard", [NBLK * P, TW], BF16, kind="Internal")
    ed_tab = [nc.dram_tensor(f"ed{l}tab", [NBLK * P, 1], BF16, kind="Internal")
              for l in range(2)]
    sc_shard = nc.dram_tensor("scshard", [NBLK, P, 1], F32, kind="Internal")
    sc_full = nc.dram_tensor("scfull", [N, 1], F32, kind="Internal",
                             addr_space="Shared")

    with tile.TileContext(nc) as tc:
        with (
            tc.tile_pool(name="const", bufs=1) as cpool,
            tc.tile_pool(name="stream", bufs=2) as spool,
            tc.tile_pool(name="idxs", bufs=2) as ipool,
            tc.tile_pool(name="zrows", bufs=2) as zpool,
            tc.tile_pool(name="ohts", bufs=2) as opool,
            tc.tile_pool(name="zown", bufs=3) as znpool,
            tc.tile_pool(name="work", bufs=3) as wpool,
            tc.tile_pool(name="ohwp", bufs=4) as ohwpool,
            tc.tile_pool(name="stage", bufs=2) as stpool,
            tc.tile_pool(name="psacc", bufs=3, space="PSUM") as psacc,
            tc.tile_pool(name="psaps", bufs=2, space="PSUM") as psaps,
            tc.tile_pool(name="pstp", bufs=1, space="PSUM") as pstp,
            tc.tile_pool(name="psproj", bufs=1, space="PSUM") as psproj,
            tc.tile_pool(name="pssc", bufs=1, space="PSUM") as pssc,
        ):
            sb = {}
            for name in ("w0big", "w1big", "b0t", "b1t", "mw0", "mw1", "mb0",
                         "iota", "iotac", "ident", "dloc", "cidx"):
                t = inp[name]
                dt = {"cidx": I32, "iota": BF16, "iotac": BF16,
                      "dloc": BF16}.get(name, F32)
                sb[name] = cpool.tile(list(t.shape), dt, tag=name, name=name)
                nc.sync.dma_start(sb[name][:], t[:])

            # ===== phase 0: shard z0 projection + ed0 table + AllGather ====
            NSTG = 8
            for sb0 in range(0, NBLK, 16):
                sbn = min(16, NBLK - sb0)
                swsh_sb = spool.tile([IN_DIM, 16 * P], F32, tag="swt",
                                     name="swsh")
                nc.sync.dma_start(swsh_sb[:, :sbn * P],
                                  inp["swtsh"][:, sb0 * P:(sb0 + sbn) * P])
                for j0 in range(0, sbn, NSTG):
                    jn = min(NSTG, sbn - j0)
                    stg = stpool.tile([P, NSTG * TW], BF16, tag="z0st",
                                      name="z0stg")
                    nc.vector.memset(stg[:], 1.0)
                    estg = stpool.tile([P, NSTG], BF16, tag="edst",
                                       name="edstg")
                    for j in range(jn):
                        bl = j0 + j
                        ps = psacc.tile([P, HID + 2], F32, tag="acc",
                                        name="z0ps")
                        nc.tensor.matmul(
                            ps[:], swsh_sb[:, bl * P:(bl + 1) * P],
                            sb["w0big"][:], start=True, stop=True)
                        if j % 2 == 0:
                            nc.scalar.copy(stg[:, j * TW:j * TW + HID + 1],
                                           ps[:, :HID + 1])
                        else:
                            nc.vector.tensor_copy(
                                stg[:, j * TW:j * TW + HID + 1],
                                ps[:, :HID + 1])
                        nc.vector.tensor_copy(estg[:, j:j + 1],
                                              ps[:, HID + 1:HID + 2])
                    r0 = (sb0 + j0) * P
                    nc.sync.dma_start(
                        z0_shard[r0:r0 + jn * P].rearrange("(j p) c -> p j c",
                                                           p=P),
                        stg[:, :jn * TW].rearrange("p (j c) -> p j c", c=TW))
                    nc.sync.dma_start(
                        ed_tab[0][r0:r0 + jn * P].rearrange(
                            "(j p) c -> p j c", p=P),
                        estg[:, :jn].rearrange("p (j c) -> p j c", c=1))
            nc.gpsimd.collective_compute(
                "AllGather", ALU.bypass,
                replica_groups=[list(range(n_cores))],
                ins=[z0_shard[:].flatten()[0:NSH * TW].opt()],
                outs=[z0_tab[0:N].flatten().opt()])

            # ======================= GAT layers ============================
            for layer in range(2):
                btile = sb["b0t"] if layer == 0 else sb["b1t"]
                tab = z0_tab if layer == 0 else z1_tab
                shard = z0_shard if layer == 0 else z1_shard
                for gi_, gs in enumerate(groups):
                    gb0, g, gc, ni = gs["b0"], gs["g"], gs["gc"], gs["ni"]
                    ic0 = gs["icol0"]
                    zr = zpool.tile([P, max_gc, TW], BF16, tag="zr", name="zr")
                    oht = opool.tile([P, max_ni * P], BF16, tag="oht",
                                     name="oht")
                    nc.sync.dma_start(oht[:, :ni * P],
                                      inp["ohT"][:, ic0 * P:(ic0 + ni) * P])
                    edc = ipool.tile([P, GG], BF16, tag="edc", name="edc")
                    nc.sync.dma_start(
                        edc[:, :g],
                        ed_tab[layer][gb0 * P:(gb0 + g) * P].rearrange(
                            "(j p) c -> p (j c)", p=P))
                    # --- gathers (per index segment) ---
                    for (s, goff_s, ln) in goff[gi_]:
                        gidx = ipool.tile([P, max(ln // 16, 1)], I16,
                                          tag="gidx", name="gidxt")
                        nc.sync.dma_start(
                            gidx[:, :ln // 16],
                            inp["gidx16"][:, goff_s:goff_s + ln // 16])
                        for (s_, cb, nch) in gs["segs"]:
                            if s_ == s:
                                break
                        assert s_ == s and nch == ln // P
                        nc.gpsimd.dma_gather(
                            out_ap=zr[:, cb - gs["segs"][0][1]:
                                      cb - gs["segs"][0][1] + nch, :],
                            in_ap=tab[s * SEGR:(s + 1) * SEGR],
                            idxs_ap=gidx[:, :ln // 16],
                            num_idxs=ln, num_idxs_reg=ln, elem_size=TW,
                            single_packet=SP)
                    # --- self-loop diagonal per block (opens psum accum) ---
                    bps = {}
                    for bi in range(g):
                        b = gb0 + bi
                        zo = znpool.tile([P, TW], BF16, tag="zo", name="zo")
                        nc.sync.dma_start(zo[:], shard[b * P:(b + 1) * P])
                        sxa = wpool.tile([P, 1], F32, tag="sxa", name="sxa")
                        nc.vector.tensor_tensor(
                            out=sxa[:], in0=zo[:, HID:HID + 1],
                            in1=edc[:, bi:bi + 1], op=ALU.add)
                        sxl = wpool.tile([P, 1], F32, tag="sxl", name="sxl")
                        nc.vector.scalar_tensor_tensor(
                            out=sxl[:], in0=sxa[:], scalar=NEG_SLOPE,
                            in1=sxa[:], op0=ALU.mult, op1=ALU.max)
                        sx = wpool.tile([P, 1], BF16, tag="sx", name="sx")
                        nc.scalar.activation(out=sx[:], in_=sxl[:],
                                             func=ACTF.Exp)
                        dg = ohwpool.tile([P, P], BF16, tag="ohw", name="dg")
                        nc.vector.scalar_tensor_tensor(
                            out=dg[:], in0=sb["iota"][:],
                            scalar=sb["iotac"][:],
                            in1=sx[:].to_broadcast([P, P]),
                            op0=ALU.is_equal, op1=ALU.mult)
                        bps[bi] = psacc.tile([P, HID + 2], F32, tag="acc",
                                             name="bps")
                        nc.tensor.matmul(
                            bps[bi][:], dg[:], zo[:, 0:HID + 2],
                            start=True, stop=(len([i for i in gs["insts"]
                                                   if i["bb"] == bi]) == 0),
                            skip_group_check=True)
                    # --- pass 1: ed per edge via one-hot matmuls ---
                    aps = psaps.tile([P, max_gc], F32, tag="aps", name="aps")
                    for inst in gs["insts"]:
                        cl = inst["cl"] - gs["segs"][0][1]
                        lc = inst["icol"] - ic0
                        nc.tensor.matmul(
                            aps[:, cl:cl + 1],
                            oht[:, lc * P:(lc + 1) * P],
                            edc[:, inst["bb"]:inst["bb"] + 1],
                            start=inst["ed_first"], stop=inst["ed_last"],
                            skip_group_check=True)
                    # --- alpha = es + ed, lrelu, exp (batched per group) ---
                    tse = wpool.tile([P, max_gc], F32, tag="tse", name="tse")
                    nc.vector.tensor_tensor(
                        out=tse[:, :gc], in0=aps[:, :gc],
                        in1=zr[:, :gc, HID:HID + 1].rearrange(
                            "p a b -> p (a b)"),
                        op=ALU.add)
                    lr = wpool.tile([P, max_gc], F32, tag="lr", name="lr")
                    nc.vector.scalar_tensor_tensor(
                        out=lr[:, :gc], in0=tse[:, :gc], scalar=NEG_SLOPE,
                        in1=tse[:, :gc], op0=ALU.mult, op1=ALU.max)
                    exc = wpool.tile([P, max_gc], BF16, tag="exc", name="exc")
                    nc.scalar.activation(out=exc[:, :gc], in_=lr[:, :gc],
                                         func=ACTF.Exp)
                    # --- pass 2: weighted one-hot + scatter matmuls ---
                    if layer == 0:
                        z1stg = stpool.tile([P, GG * TW], BF16, tag="z1st",
                                            name="z1stg")
                        nc.vector.memset(z1stg[:], 1.0)
                        ed1stg = stpool.tile([P, GG], BF16, tag="ed1st",
                                             name="ed1stg")
                    else:
                        scstg = stpool.tile([1, GG * P], F32, tag="scst",
                                            name="scstg")
                    for inst in gs["insts"]:
                        cl = inst["cl"] - gs["segs"][0][1]
                        ohw = ohwpool.tile([P, P], BF16, tag="ohw",
                                           name="ohw")
                        nc.vector.scalar_tensor_tensor(
                            out=ohw[:], in0=sb["iota"][:],
                            scalar=sb["dloc"][:, inst["icol"]:
                                              inst["icol"] + 1],
                            in1=exc[:, cl:cl + 1].to_broadcast([P, P]),
                            op0=ALU.is_equal, op1=ALU.mult)
                        nc.tensor.matmul(
                            bps[inst["bb"]][:], ohw[:],
                            zr[:, cl:cl + 1, 0:HID + 2].squeeze(),
                            start=False, stop=inst["acc_last"],
                            skip_group_check=True)
                    # --- epilogues (all chunks of the group are done) ---
                    for bi in range(g):
                        b = gb0 + bi
                        pb = bps[bi]
                        rc = wpool.tile([P, 1], F32, tag="rc", name="rc")
                        nc.vector.reciprocal(rc[:], pb[:, HID + 1:HID + 2])
                        y = wpool.tile([P, HID], F32, tag="y", name="y")
                        nc.vector.scalar_tensor_tensor(
                            out=y[:], in0=pb[:, :HID], scalar=rc[:],
                            in1=btile[:], op0=ALU.mult, op1=ALU.add)
                        e_t = wpool.tile([P, HID], F32, tag="e_t", name="e_t")
                        r_t = wpool.tile([P, HID], F32, tag="r_t", name="r_t")
                        nc.scalar.activation(out=e_t[:], in_=y[:],
                                             func=ACTF.Exp)
                        nc.scalar.activation(out=r_t[:], in_=y[:],
                                             func=ACTF.Relu)
                        hp1 = wpool.tile([P, HID], F32, tag="hp1", name="hp1")
                        nc.vector.scalar_tensor_tensor(
                            out=hp1[:], in0=e_t[:], scalar=1.0, in1=r_t[:],
                            op0=ALU.min, op1=ALU.add)      # elu(y) + 1
                        tp = pstp.tile([P, P], F32, tag="tp", name="tp")
                        nc.tensor.transpose(tp[:], hp1[:], sb["ident"][:])
                        hT = wpool.tile([P, HID], F32, tag="hT", name="hT")
                        nc.scalar.activation(out=hT[:], in_=tp[:],
                                             func=ACTF.Copy, bias=-1.0)
                        if layer == 0:
                            zps = psproj.tile([P, HID + 2], F32, tag="proj",
                                              name="zps")
                            nc.tensor.matmul(zps[:], hT[:], sb["w1big"][:],
                                             start=True, stop=True,
                                             skip_group_check=True)
                            nc.scalar.copy(
                                z1stg[:, bi * TW:bi * TW + HID + 1],
                                zps[:, :HID + 1])
                            nc.vector.tensor_copy(
                                ed1stg[:, bi:bi + 1],
                                zps[:, HID + 1:HID + 2])
                        else:
                            mps = psproj.tile([P, HID], F32, tag="proj",
                                              name="mps")
                            nc.tensor.matmul(mps[:], sb["mw0"][:], hT[:],
                                             start=True, stop=True,
                                             skip_group_check=True)
                            m1 = wpool.tile([P, HID], F32, tag="m1", name="m1")
                            nc.scalar.activation(out=m1[:], in_=mps[:],
                                                 func=ACTF.Relu,
                                                 bias=sb["mb0"][:])
                            sps = pssc.tile([1, P], F32, tag="sc",
                                            name="sps")
                            nc.tensor.matmul(sps[:], sb["mw1"][:], m1[:],
                                             start=True, stop=True,
                                             skip_group_check=True)
                            nc.scalar.copy(scstg[:, bi * P:(bi + 1) * P],
                                           sps[:])
                    if layer == 0:
                        nc.sync.dma_start(
                            z1_shard[gb0 * P:(gb0 + g) * P].rearrange(
                                "(j p) c -> p j c", p=P),
                            z1stg[:, :g * TW].rearrange("p (j c) -> p j c",
                                                        c=TW))
                        nc.sync.dma_start(
                            ed_tab[1][gb0 * P:(gb0 + g) * P].rearrange(
                                "(j p) c -> p j c", p=P),
                            ed1stg[:, :g].rearrange("p (j c) -> p j c", c=1))
                    else:
                        nc.sync.dma_start(sc_shard[gb0:gb0 + g],
                                          scstg[:, :g * P])
                if layer == 0:
                    nc.gpsimd.collective_compute(
                        "AllGather", ALU.bypass,
                        replica_groups=[list(range(n_cores))],
                        ins=[z1_shard[:].flatten()[0:NSH * TW].opt()],
                        outs=[z1_tab[0:N].flatten().opt()])

            # ================= scores + candidate softmax ==================
            nc.gpsimd.collective_compute(
                "AllGather", ALU.bypass,
                replica_groups=[list(range(n_cores))],
                ins=[sc_shard[:].flatten()[0:NSH].opt()],
                outs=[sc_full[:].flatten().opt()])
            scg = wpool.tile([P, CC], F32, tag="scg", name="scg")
            for c in range(CC):
                nc.gpsimd.indirect_dma_start(
                    out=scg[:, c:c + 1], out_offset=None, in_=sc_full[:],
                    in_offset=IndirectOffsetOnAxis(
                        ap=sb["cidx"][:, c:c + 1], axis=0))
            NG = CC // VM
            pex = wpool.tile([P, CC], F32, tag="pex", name="pex")
            nc.scalar.activation(out=pex[:], in_=scg[:], func=ACTF.Exp)
            ssum = wpool.tile([P, NG], F32, tag="ssum", name="ssum")
            nc.vector.tensor_reduce(
                out=ssum[:], in_=pex[:].rearrange("p (g v) -> p g v", v=VM),
                axis=mybir.AxisListType.X, op=ALU.add)
            rcg = wpool.tile([P, NG], F32, tag="rcg", name="rcg")
            nc.vector.reciprocal(rcg[:], ssum[:])
            pi = wpool.tile([P, CC], F32, tag="pi", name="pi")
            for g_ in range(NG):
                nc.vector.tensor_scalar(
                    out=pi[:, g_ * VM:(g_ + 1) * VM],
                    in0=pex[:, g_ * VM:(g_ + 1) * VM],
                    scalar1=rcg[:, g_:g_ + 1], scalar2=0.0,
                    op0=ALU.mult, op1=ALU.add)
            nc.sync.dma_start(out_t[:], pi[:])

    return nc


# ------------------------------------------------------------------- kernel
def kernel(**inputs):
    n_cores = 8
    meta, in_maps = _prep_inputs(inputs, n_cores)
    nc = build(meta, n_cores)
    nc.compile()
    res = run_bass_kernel_spmd(
        nc, in_maps, core_ids=list(range(n_cores)),
        trace=bool(int(os.environ.get("KERNEL_TRACE", "0"))))
    kernel.last_results = res
    kernel.last_meta = meta
    VM = meta["VM"]
    outs = [res.results[c]["out"].reshape(-1, VM) for c in range(n_cores)]
    return np.concatenate(outs, axis=0).astype(np.float32)


# revision 9
# speedup vs baseline: 1.7678x; 1.0099x over previous
"""Trainium2 Bass kernel: 2-layer GAT (PyG GATConv, heads=1) + per-node actor
MLP + candidate softmax, SPMD across 8 NeuronCores.

Strategy (dst-sharded data parallel):
  - Symmetrized edges (self loops handled separately), partitioned by dst
    across 8 cores, grouped into 128-dst blocks, GG blocks per gather group.
    Per (group, segment) the edges are packed block-major into one padded
    run of 128-edge chunks; a chunk may span adjacent blocks, handled by
    per-(chunk, block) matmul instances.
  - Node table per layer: bf16 [z(128) | e_src | 1.0 | pad] rows (512B).
    Per edge, dma_gather pulls the src row (segment-relative int16 idx).
  - Host precomputes the transposed one-hot (dst-on-partitions) per
    instance, streamed bf16; one matmul per instance gathers e_dst onto
    edge partitions. alpha = es + ed batched per group: DVE add + DVE
    leaky-relu + one Act exp (keeps Act inside one act-table set).
  - ohw[e,dst] = (iota==d_local)*ex built in one bf16 DVE op per instance;
    one PE matmul per instance accumulates numerator AND denominator
    (table's ones column) into the block psum. Self-loop contributions are
    added per block via a diagonal matmul from the local z-shard staging.
  - Epilogue per block: h = elu(num/den + b) (+1 trick), PE transpose,
    projection to next layer's table row + the shard-local ed table.
    Phase 0 and the layer boundary AllGather the bf16 node tables.
  - Scores are per-node scalars -> AllGather 400KB -> candidate gather +
    grouped softmax over vm=16, sharded over decisions.
"""

import math
import os
import sys

sys.path.insert(0, "/opt/trn_rl_repo")

import ml_dtypes
import numpy as np

import concourse.bass as bass
import concourse.mybir as mybir
import concourse.tile as tile
from concourse import bacc
from concourse.bass import IndirectOffsetOnAxis
from concourse.bass_utils import run_bass_kernel_spmd

F32 = mybir.dt.float32
I32 = mybir.dt.int32
I16 = mybir.dt.int16
BF16 = mybir.dt.bfloat16
ALU = mybir.AluOpType
ACTF = mybir.ActivationFunctionType
BF = ml_dtypes.bfloat16

NEG_SLOPE = 0.2
P = 128
SEGR = 32768          # table rows per int16-addressable segment
TW = 256              # bf16 table row: z(128) | es | 1.0 | pad  (512B)
GG = 3                # blocks per gather group


# ----------------------------------------------------------------- host prep
def _schedule(edge_index, N, n_cores):
    """Common chunk/instance schedule + per-core index arrays."""
    NSH = N // n_cores
    NBLK = math.ceil(NSH / P)
    NSEG = math.ceil(N / SEGR)
    e0 = edge_index[0].astype(np.int64)
    e1 = edge_index[1].astype(np.int64)
    src = np.concatenate([e0, e1])
    dst = np.concatenate([e1, e0])

    # bucket edges: per core, per block, per segment (src-sorted)
    percore = []
    for c in range(n_cores):
        m = (dst >= c * NSH) & (dst < (c + 1) * NSH)
        s_c, d_c = src[m], dst[m] - c * NSH
        o = np.lexsort((s_c, d_c // P))
        s_c, d_c = s_c[o], d_c[o]
        blk = d_c // P
        bs = np.searchsorted(blk, np.arange(NBLK), side="left")
        be = np.searchsorted(blk, np.arange(NBLK), side="right")
        per_blk = []
        for b in range(NBLK):
            sb_, db_ = s_c[bs[b]:be[b]], d_c[bs[b]:be[b]]
            seg = sb_ >> 15
            segs = []
            for s in range(NSEG):
                sm = seg == s
                segs.append((sb_[sm], db_[sm]))
            per_blk.append(segs)
        percore.append(per_blk)

    # common per (group, seg) padded run lengths (shared by all cores) and
    # per (group, seg, block) edge counts per core to derive instance spans
    ngrp = math.ceil(NBLK / GG)
    run_len = np.zeros((ngrp, NSEG), dtype=np.int64)   # padded (x128)
    for gi in range(ngrp):
        b0 = gi * GG
        g = min(GG, NBLK - b0)
        for s in range(NSEG):
            mx = 0
            for c in range(n_cores):
                tot = sum(len(percore[c][b0 + bb][s][0]) for bb in range(g))
                mx = max(mx, tot)
            run_len[gi, s] = math.ceil(mx / P) * P if mx else 0

    # groups meta: per group, per seg: chunk col base; chunk count
    groups = []          # (b0, g, segs=[(s, cb_chunk, nch)], gc)
    kk = 0
    for gi in range(ngrp):
        b0 = gi * GG
        g = min(GG, NBLK - b0)
        segs = []
        for s in range(NSEG):
            nch = int(run_len[gi, s]) // P
            segs.append((s, kk, nch))
            kk += nch
        gc = sum(x[2] for x in segs)
        groups.append((b0, g, segs))
    K = kk

    # per-core: index streams, per-instance dloc and instance schedule.
    # The instance schedule (which blocks each chunk touches) must be
    # IDENTICAL across cores (SPMD single program): merge spans over cores.
    # For each (group, seg, chunk) the set of possibly-touching blocks is
    # derived from per-core block spans; union over cores.
    inst_sets = [dict() for _ in range(ngrp)]   # (s, chunk) -> set(blocks)
    percore_edges = []
    for c in range(n_cores):
        ge = []
        for gi, (b0, g, segs) in enumerate(groups):
            for (s, cb, nch) in segs:
                if nch == 0:
                    continue
                cap = nch * P
                sp = np.zeros(cap, dtype=np.int64)
                dp = np.full(cap, 200.0, dtype=np.float32)
                bl = np.full(cap, -1, dtype=np.int64)
                off = 0
                for bb in range(g):
                    sb_, db_ = percore[c][b0 + bb][s]
                    ns = len(sb_)
                    sp[off:off + ns] = sb_ - s * SEGR
                    dp[off:off + ns] = (db_ - (b0 + bb) * P)
                    bl[off:off + ns] = bb
                    off += ns
                for k in range(nch):
                    touched = set(bl[k * P:(k + 1) * P].tolist()) - {-1}
                    key = (s, cb + k)
                    inst_sets[gi].setdefault(key, set()).update(touched)
                ge.append((gi, s, cb, nch, sp, dp, bl))
        percore_edges.append(ge)

    # canonical instance order per group: seg-major, chunk-major, block asc
    inst_meta = []       # per group: list of (s, chunk_col, bb)
    for gi, (b0, g, segs) in enumerate(groups):
        il = []
        for (s, cb, nch) in segs:
            for k in range(nch):
                bbs = sorted(inst_sets[gi].get((s, cb + k), set()))
                if not bbs:
                    bbs = [g - 1]          # dummy all-pad chunk
                for bb in bbs:
                    il.append((s, cb + k, bb))
        inst_meta.append(il)
    I = sum(len(il) for il in inst_meta)

    # build per-group kernel schedules
    gsched = []
    icol = 0
    for gi, (b0, g, segs) in enumerate(groups):
        il = inst_meta[gi]
        # ed-matmul first/last per chunk; acc last per block
        by_chunk = {}
        by_blk = {}
        insts = []
        for j, (s, cl, bb) in enumerate(il):
            by_chunk.setdefault(cl, []).append(j)
            by_blk.setdefault(bb, []).append(j)
        for j, (s, cl, bb) in enumerate(il):
            insts.append(dict(
                cl=cl, bb=bb, icol=icol + j,
                ed_first=(j == by_chunk[cl][0]),
                ed_last=(j == by_chunk[cl][-1]),
                acc_last=(j == by_blk[bb][-1])))
        gc = sum(x[2] for x in segs)
        gsched.append(dict(b0=b0, g=g, segs=segs, gc=gc, insts=insts,
                           icol0=icol, ni=len(il)))
        icol += len(il)
    assert icol == I

    meta = dict(NSH=NSH, NBLK=NBLK, NSEG=NSEG, K=K, I=I, groups=gsched)

    # per-core tensors
    out = []
    for c in range(n_cores):
        dlocI = np.full((P, I), 200.0, dtype=np.float32)
        gw_parts, goff = [], []
        go = 0
        chunk_dp = {}
        for (gi, s, cb, nch, sp, dp, bl) in percore_edges[c]:
            for k in range(nch):
                chunk_dp[(gi, s, cb + k)] = (dp[k * P:(k + 1) * P],
                                             bl[k * P:(k + 1) * P])
        for gi, (b0, g, segs) in enumerate(groups):
            slens = []
            for (gi2, s, cb, nch, sp, dp, bl) in percore_edges[c]:
                if gi2 != gi:
                    continue
                w = sp.reshape(-1, 16).T
                gw_parts.append(np.tile(w, (8, 1)).astype(np.int16))
                slens.append((s, go, len(sp)))
                go += len(sp) // 16
            goff.append(slens)
            for inst in gsched[gi]["insts"]:
                cl, bb, ic = inst["cl"], inst["bb"], inst["icol"]
                # find seg of this chunk
                for (s, cb, nch) in segs:
                    if cb <= cl < cb + nch:
                        break
                dpk, blk_ = chunk_dp.get((gi, s, cl), (None, None))
                if dpk is None:
                    continue
                v = np.where(blk_ == bb, dpk, 200.0)
                dlocI[:, ic] = v
        ohT = (np.arange(P, dtype=np.float32)[:, None, None]
               == dlocI.T[None, :, :])
        out.append(dict(
            gidx16=np.concatenate(gw_parts, axis=1),
            dloc=dlocI.astype(BF),
            ohT=np.ascontiguousarray(ohT.reshape(P, I * P)).astype(BF)))
    meta["goff"] = goff
    return meta, out


def _prep_inputs(inputs, n_cores=8):
    N, IN_DIM = inputs["state_wf"].shape
    HID = inputs["W0"].shape[1]
    VM = 16
    B = inputs["candidate_task_index"].shape[0] // VM
    meta, per_core_e = _schedule(inputs["edge_index"], N, n_cores)
    meta.update(N=N, IN_DIM=IN_DIM, HID=HID, VM=VM, B=B,
                NPAD=math.ceil(N / P) * P)

    f = lambda x: np.asarray(x, dtype=np.float32)
    W0, W1 = f(inputs["W0"]), f(inputs["W1"])
    w0big = np.concatenate(
        [W0, (W0 @ f(inputs["a_src0"]))[:, None],
         (W0 @ f(inputs["a_dst0"]))[:, None]], axis=1)
    w1big = np.concatenate(
        [W1, (W1 @ f(inputs["a_src1"]))[:, None],
         (W1 @ f(inputs["a_dst1"]))[:, None]], axis=1)
    swt = np.zeros((IN_DIM, meta["NPAD"]), dtype=np.float32)
    swt[:, :N] = f(inputs["state_wf"]).T
    NSH, NBLK = meta["NSH"], meta["NBLK"]
    common = dict(
        w0big=w0big.astype(np.float32),
        w1big=w1big.astype(np.float32),
        b0t=np.tile(f(inputs["b0"])[None, :], (P, 1)).astype(np.float32),
        b1t=np.tile(f(inputs["b1"])[None, :], (P, 1)).astype(np.float32),
        mw0=f(inputs["mW0"]),
        mw1=f(inputs["mW1"]).reshape(HID, 1),
        mb0=f(inputs["mb0"]).reshape(HID, 1),
        iota=np.tile(np.arange(P, dtype=np.float32)[None, :],
                     (P, 1)).astype(BF),
        iotac=np.arange(P, dtype=np.float32).reshape(P, 1).astype(BF),
        ident=np.eye(P, dtype=np.float32),
    )
    cand = inputs["candidate_task_index"].astype(np.int64)
    CPC = (B // n_cores) * VM
    CC = CPC // P
    meta["CC"] = CC
    in_maps = []
    for c in range(n_cores):
        m = dict(common)
        m.update(per_core_e[c])
        m["swtsh"] = np.ascontiguousarray(
            np.pad(swt[:, c * NSH:(c + 1) * NSH],
                   ((0, 0), (0, NBLK * P - NSH))))
        m["cidx"] = cand[c * CPC:(c + 1) * CPC].reshape(P, CC).astype(np.int32)
        in_maps.append(m)
    return meta, in_maps


# ------------------------------------------------------------------ builder
def build(meta, n_cores=8):
    NSH, NBLK, NSEG = meta["NSH"], meta["NBLK"], meta["NSEG"]
    K, I, groups = meta["K"], meta["I"], meta["groups"]
    N, IN_DIM, HID = meta["N"], meta["IN_DIM"], meta["HID"]
    NPAD, CC, VM = meta["NPAD"], meta["CC"], meta["VM"]
    goff = meta["goff"]
    NTROW = NSEG * SEGR
    IWG = sum(ln // 16 for slens in goff for (_, _, ln) in slens)
    max_gc = max(gs["gc"] for gs in groups)
    max_ni = max(gs["ni"] for gs in groups)

    SP = bool(int(os.environ.get("KERNEL_SP", "0")))
    nc = bacc.Bacc("TRN2", target_bir_lowering=False, debug=False,
                   enable_asserts=False, num_devices=n_cores)

    inp = {}
    for name, shape, dt in [
        ("swtsh", [IN_DIM, NBLK * P], F32),
        ("w0big", [IN_DIM, HID + 2], F32), ("w1big", [HID, HID + 2], F32),
        ("b0t", [P, HID], F32), ("b1t", [P, HID], F32),
        ("mw0", [HID, HID], F32), ("mw1", [HID, 1], F32),
        ("mb0", [HID, 1], F32), ("iota", [P, P], BF16),
        ("iotac", [P, 1], BF16), ("ident", [P, P], F32),
        ("dloc", [P, I], BF16), ("ohT", [P, I * P], BF16),
        ("gidx16", [P, IWG], I16), ("cidx", [P, CC], I32),
    ]:
        inp[name] = nc.dram_tensor(name, shape, dt, kind="ExternalInput")
    out_t = nc.dram_tensor("out", [P, CC], F32, kind="ExternalOutput")

    z0_shard = nc.dram_tensor("z0shard", [NBLK * P, TW], BF16, kind="Internal")
    z0_tab = nc.dram_tensor("z0tab", [NTROW, TW], BF16, kind="Internal",
                            addr_space="Shared")
    z1_tab = nc.dram_tensor("z1tab", [NTROW, TW], BF16, kind="Internal",
                            addr_space="Shared")
    z1_shard = nc.dram_tensor("z1shard", [NBLK * P, TW], BF16, kind="Internal")
    ed_tab = [nc.dram_tensor(f"ed{l}tab", [NBLK * P, 1], BF16, kind="Internal")
              for l in range(2)]
    sc_shard = nc.dram_tensor("scshard", [NBLK, P, 1], F32, kind="Internal")
    sc_full = nc.dram_tensor("scfull", [N, 1], F32, kind="Internal",
                             addr_space="Shared")

    with tile.TileContext(nc) as tc:
        with (
            tc.tile_pool(name="const", bufs=1) as cpool,
            tc.tile_pool(name="stream", bufs=2) as spool,
            tc.tile_pool(name="idxs", bufs=2) as ipool,
            tc.tile_pool(name="zrows", bufs=3) as zpool,
            tc.tile_pool(name="ohts", bufs=3) as opool,
            tc.tile_pool(name="zown", bufs=3) as znpool,
            tc.tile_pool(name="work", bufs=3) as wpool,
            tc.tile_pool(name="ohwp", bufs=4) as ohwpool,
            tc.tile_pool(name="stage", bufs=2) as stpool,
            tc.tile_pool(name="psacc", bufs=3, space="PSUM") as psacc,
            tc.tile_pool(name="psaps", bufs=2, space="PSUM") as psaps,
            tc.tile_pool(name="pstp", bufs=1, space="PSUM") as pstp,
            tc.tile_pool(name="psproj", bufs=1, space="PSUM") as psproj,
            tc.tile_pool(name="pssc", bufs=1, space="PSUM") as pssc,
        ):
            sb = {}
            for name in ("w0big", "w1big", "b0t", "b1t", "mw0", "mw1", "mb0",
                         "iota", "iotac", "ident", "dloc", "cidx"):
                t = inp[name]
                dt = {"cidx": I32, "iota": BF16, "iotac": BF16,
                      "dloc": BF16}.get(name, F32)
                sb[name] = cpool.tile(list(t.shape), dt, tag=name, name=name)
                nc.sync.dma_start(sb[name][:], t[:])

            # ===== phase 0: shard z0 projection + ed0 table + AllGather ====
            NSTG = 8
            for sb0 in range(0, NBLK, 16):
                sbn = min(16, NBLK - sb0)
                swsh_sb = spool.tile([IN_DIM, 16 * P], F32, tag="swt",
                                     name="swsh")
                nc.sync.dma_start(swsh_sb[:, :sbn * P],
                                  inp["swtsh"][:, sb0 * P:(sb0 + sbn) * P])
                for j0 in range(0, sbn, NSTG):
                    jn = min(NSTG, sbn - j0)
                    stg = stpool.tile([P, NSTG * TW], BF16, tag="z0st",
                                      name="z0stg")
                    nc.vector.memset(stg[:], 1.0)
                    estg = stpool.tile([P, NSTG], BF16, tag="edst",
                                       name="edstg")
                    for j in range(jn):
                        bl = j0 + j
                        ps = psacc.tile([P, HID + 2], F32, tag="acc",
                                        name="z0ps")
                        nc.tensor.matmul(
                            ps[:], swsh_sb[:, bl * P:(bl + 1) * P],
                            sb["w0big"][:], start=True, stop=True)
                        if j % 2 == 0:
                            nc.scalar.copy(stg[:, j * TW:j * TW + HID + 1],
                                           ps[:, :HID + 1])
                        else:
                            nc.vector.tensor_copy(
                                stg[:, j * TW:j * TW + HID + 1],
                                ps[:, :HID + 1])
                        nc.vector.tensor_copy(estg[:, j:j + 1],
                                              ps[:, HID + 1:HID + 2])
                    r0 = (sb0 + j0) * P
                    nc.sync.dma_start(
                        z0_shard[r0:r0 + jn * P].rearrange("(j p) c -> p j c",
                                                           p=P),
                        stg[:, :jn * TW].rearrange("p (j c) -> p j c", c=TW))
                    nc.sync.dma_start(
                        ed_tab[0][r0:r0 + jn * P].rearrange(
                            "(j p) c -> p j c", p=P),
                        estg[:, :jn].rearrange("p (j c) -> p j c", c=1))
            nc.gpsimd.collective_compute(
                "AllGather", ALU.bypass,
                replica_groups=[list(range(n_cores))],
                ins=[z0_shard[:].flatten()[0:NSH * TW].opt()],
                outs=[z0_tab[0:N].flatten().opt()])

            # ======================= GAT layers ============================
            for layer in range(2):
                btile = sb["b0t"] if layer == 0 else sb["b1t"]
                tab = z0_tab if layer == 0 else z1_tab
                shard = z0_shard if layer == 0 else z1_shard
                for gi_, gs in enumerate(groups):
                    gb0, g, gc, ni = gs["b0"], gs["g"], gs["gc"], gs["ni"]
                    ic0 = gs["icol0"]
                    zr = zpool.tile([P, max_gc, TW], BF16, tag="zr", name="zr")
                    oht = opool.tile([P, max_ni * P], BF16, tag="oht",
                                     name="oht")
                    nc.sync.dma_start(oht[:, :ni * P],
                                      inp["ohT"][:, ic0 * P:(ic0 + ni) * P])
                    edc = ipool.tile([P, GG], BF16, tag="edc", name="edc")
                    nc.sync.dma_start(
                        edc[:, :g],
                        ed_tab[layer][gb0 * P:(gb0 + g) * P].rearrange(
                            "(j p) c -> p (j c)", p=P))
                    # --- gathers (per index segment) ---
                    for (s, goff_s, ln) in goff[gi_]:
                        gidx = ipool.tile([P, max(ln // 16, 1)], I16,
                                          tag="gidx", name="gidxt")
                        nc.sync.dma_start(
                            gidx[:, :ln // 16],
                            inp["gidx16"][:, goff_s:goff_s + ln // 16])
                        for (s_, cb, nch) in gs["segs"]:
                            if s_ == s:
                                break
                        assert s_ == s and nch == ln // P
                        nc.gpsimd.dma_gather(
                            out_ap=zr[:, cb - gs["segs"][0][1]:
                                      cb - gs["segs"][0][1] + nch, :],
                            in_ap=tab[s * SEGR:(s + 1) * SEGR],
                            idxs_ap=gidx[:, :ln // 16],
                            num_idxs=ln, num_idxs_reg=ln, elem_size=TW,
                            single_packet=SP)
                    # --- self-loop diagonal per block (opens psum accum) ---
                    bps = {}
                    for bi in range(g):
                        b = gb0 + bi
                        zo = znpool.tile([P, TW], BF16, tag="zo", name="zo")
                        nc.sync.dma_start(zo[:], shard[b * P:(b + 1) * P])
                        sxa = wpool.tile([P, 1], F32, tag="sxa", name="sxa")
                        nc.vector.tensor_tensor(
                            out=sxa[:], in0=zo[:, HID:HID + 1],
                            in1=edc[:, bi:bi + 1], op=ALU.add)
                        sxl = wpool.tile([P, 1], F32, tag="sxl", name="sxl")
                        nc.vector.scalar_tensor_tensor(
                            out=sxl[:], in0=sxa[:], scalar=NEG_SLOPE,
                            in1=sxa[:], op0=ALU.mult, op1=ALU.max)
                        sx = wpool.tile([P, 1], BF16, tag="sx", name="sx")
                        nc.scalar.activation(out=sx[:], in_=sxl[:],
                                             func=ACTF.Exp)
                        dg = ohwpool.tile([P, P], BF16, tag="ohw", name="dg")
                        nc.vector.scalar_tensor_tensor(
                            out=dg[:], in0=sb["iota"][:],
                            scalar=sb["iotac"][:],
                            in1=sx[:].to_broadcast([P, P]),
                            op0=ALU.is_equal, op1=ALU.mult)
                        bps[bi] = psacc.tile([P, HID + 2], F32, tag="acc",
                                             name="bps")
                        nc.tensor.matmul(
                            bps[bi][:], dg[:], zo[:, 0:HID + 2],
                            start=True, stop=(len([i for i in gs["insts"]
                                                   if i["bb"] == bi]) == 0),
                            skip_group_check=True)
                    # --- pass 1: ed per edge via one-hot matmuls ---
                    aps = psaps.tile([P, max_gc], F32, tag="aps", name="aps")
                    for inst in gs["insts"]:
                        cl = inst["cl"] - gs["segs"][0][1]
                        lc = inst["icol"] - ic0
                        nc.tensor.matmul(
                            aps[:, cl:cl + 1],
                            oht[:, lc * P:(lc + 1) * P],
                            edc[:, inst["bb"]:inst["bb"] + 1],
                            start=inst["ed_first"], stop=inst["ed_last"],
                            skip_group_check=True)
                    # --- alpha = es + ed, lrelu, exp (batched per group) ---
                    tse = wpool.tile([P, max_gc], F32, tag="tse", name="tse")
                    nc.vector.tensor_tensor(
                        out=tse[:, :gc], in0=aps[:, :gc],
                        in1=zr[:, :gc, HID:HID + 1].rearrange(
                            "p a b -> p (a b)"),
                        op=ALU.add)
                    lr = wpool.tile([P, max_gc], F32, tag="lr", name="lr")
                    nc.vector.scalar_tensor_tensor(
                        out=lr[:, :gc], in0=tse[:, :gc], scalar=NEG_SLOPE,
                        in1=tse[:, :gc], op0=ALU.mult, op1=ALU.max)
                    exc = wpool.tile([P, max_gc], BF16, tag="exc", name="exc")
                    nc.scalar.activation(out=exc[:, :gc], in_=lr[:, :gc],
                                         func=ACTF.Exp)
                    # --- pass 2: weighted one-hot + scatter matmuls ---
                    if layer == 0:
                        z1stg = stpool.tile([P, GG * TW], BF16, tag="z1st",
                                            name="z1stg")
                        nc.vector.memset(z1stg[:], 1.0)
                        ed1stg = stpool.tile([P, GG], BF16, tag="ed1st",
                                             name="ed1stg")
                    else:
                        scstg = stpool.tile([1, GG * P], F32, tag="scst",
                                            name="scstg")
                    for inst in gs["insts"]:
                        cl = inst["cl"] - gs["segs"][0][1]
                        ohw = ohwpool.tile([P, P], BF16, tag="ohw",
                                           name="ohw")
                        nc.vector.scalar_tensor_tensor(
                            out=ohw[:], in0=sb["iota"][:],
                            scalar=sb["dloc"][:, inst["icol"]:
                                              inst["icol"] + 1],
                            in1=exc[:, cl:cl + 1].to_broadcast([P, P]),
                            op0=ALU.is_equal, op1=ALU.mult)
                        nc.tensor.matmul(
                            bps[inst["bb"]][:], ohw[:],
                            zr[:, cl:cl + 1, 0:HID + 2].squeeze(),
                            start=False, stop=inst["acc_last"],
                            skip_group_check=True)
                    # --- epilogues (all chunks of the group are done) ---
                    for bi in range(g):
                        b = gb0 + bi
                        pb = bps[bi]
                        rc = wpool.tile([P, 1], F32, tag="rc", name="rc")
                        nc.vector.reciprocal(rc[:], pb[:, HID + 1:HID + 2])
                        y = wpool.tile([P, HID], F32, tag="y", name="y")
                        nc.vector.scalar_tensor_tensor(
                            out=y[:], in0=pb[:, :HID], scalar=rc[:],
                            in1=btile[:], op0=ALU.mult, op1=ALU.add)
                        e_t = wpool.tile([P, HID], F32, tag="e_t", name="e_t")
                        r_t = wpool.tile([P, HID], F32, tag="r_t", name="r_t")
                        nc.scalar.activation(out=e_t[:], in_=y[:],
                                             func=ACTF.Exp)
                        nc.scalar.activation(out=r_t[:], in_=y[:],
                                             func=ACTF.Relu)
                        hp1 = wpool.tile([P, HID], F32, tag="hp1", name="hp1")
                        nc.vector.scalar_tensor_tensor(
                            out=hp1[:], in0=e_t[:], scalar=1.0, in1=r_t[:],
                            op0=ALU.min, op1=ALU.add)      # elu(y) + 1
                        tp = pstp.tile([P, P], F32, tag="tp", name="tp")
                        nc.tensor.transpose(tp[:], hp1[:], sb["ident"][:])
                        hT = wpool.tile([P, HID], F32, tag="hT", name="hT")
                        nc.scalar.activation(out=hT[:], in_=tp[:],
                                             func=ACTF.Copy, bias=-1.0)
                        if layer == 0:
                            zps = psproj.tile([P, HID + 2], F32, tag="proj",
                                              name="zps")
                            nc.tensor.matmul(zps[:], hT[:], sb["w1big"][:],
                                             start=True, stop=True,
                                             skip_group_check=True)
                            nc.scalar.copy(
                                z1stg[:, bi * TW:bi * TW + HID + 1],
                                zps[:, :HID + 1])
                            nc.vector.tensor_copy(
                                ed1stg[:, bi:bi + 1],
                                zps[:, HID + 1:HID + 2])
                        else:
                            mps = psproj.tile([P, HID], F32, tag="proj",
                                              name="mps")
                            nc.tensor.matmul(mps[:], sb["mw0"][:], hT[:],
                                             start=True, stop=True,
                                             skip_group_check=True)
                            m1 = wpool.tile([P, HID], F32, tag="m1", name="m1")
                            nc.scalar.activation(out=m1[:], in_=mps[:],
                                                 func=ACTF.Relu,
                                                 bias=sb["mb0"][:])
                            sps = pssc.tile([1, P], F32, tag="sc",
                                            name="sps")
                            nc.tensor.matmul(sps[:], sb["mw1"][:], m1[:],
                                             start=True, stop=True,
                                             skip_group_check=True)
                            nc.scalar.copy(scstg[:, bi * P:(bi + 1) * P],
                                           sps[:])
                    if layer == 0:
                        nc.sync.dma_start(
                            z1_shard[gb0 * P:(gb0 + g) * P].rearrange(
                                "(j p) c -> p j c", p=P),
                            z1stg[:, :g * TW].rearrange("p (j c) -> p j c",
                                                        c=TW))
                        nc.sync.dma_start(
                            ed_tab[1][gb0 * P:(gb0 + g) * P].rearrange(
                                "(j p) c -> p j c", p=P),
                            ed1stg[:, :g].rearrange("p (j c) -> p j c", c=1))
                    else:
                        nc.sync.dma_start(sc_shard[gb0:gb0 + g],
                                          scstg[:, :g * P])
                if layer == 0:
                    nc.gpsimd.collective_compute(
                        "AllGather", ALU.bypass,
                        replica_groups=[list(range(n_cores))],
                        ins=[z1_shard[:].flatten()[0:NSH * TW].opt()],
                        outs=[z1_tab[0:N].flatten().opt()])

            # ================= scores + candidate softmax ==================
            nc.gpsimd.collective_compute(
                "AllGather", ALU.bypass,
                replica_groups=[list(range(n_cores))],
                ins=[sc_shard[:].flatten()[0:NSH].opt()],
                outs=[sc_full[:].flatten().opt()])
            scg = wpool.tile([P, CC], F32, tag="scg", name="scg")
            for c in range(CC):
                nc.gpsimd.indirect_dma_start(
                    out=scg[:, c:c + 1], out_offset=None, in_=sc_full[:],
                    in_offset=IndirectOffsetOnAxis(
                        ap=sb["cidx"][:, c:c + 1], axis=0))
            NG = CC // VM
            pex = wpool.tile([P, CC], F32, tag="pex", name="pex")
            nc.scalar.activation(out=pex[:], in_=scg[:], func=ACTF.Exp)
            ssum = wpool.tile([P, NG], F32, tag="ssum", name="ssum")
            nc.vector.tensor_reduce(
                out=ssum[:], in_=pex[:].rearrange("p (g v) -> p g v", v=VM),
                axis=mybir.AxisListType.X, op=ALU.add)
            rcg = wpool.tile([P, NG], F32, tag="rcg", name="rcg")
            nc.vector.reciprocal(rcg[:], ssum[:])
            pi = wpool.tile([P, CC], F32, tag="pi", name="pi")
            for g_ in range(NG):
                nc.vector.tensor_scalar(
                    out=pi[:, g_ * VM:(g_ + 1) * VM],
                    in0=pex[:, g_ * VM:(g_ + 1) * VM],
                    scalar1=rcg[:, g_:g_ + 1], scalar2=0.0,
                    op0=ALU.mult, op1=ALU.add)
            nc.sync.dma_start(out_t[:], pi[:])

    return nc


# ------------------------------------------------------------------- kernel
def kernel(**inputs):
    n_cores = 8
    meta, in_maps = _prep_inputs(inputs, n_cores)
    nc = build(meta, n_cores)
    nc.compile()
    res = run_bass_kernel_spmd(
        nc, in_maps, core_ids=list(range(n_cores)),
        trace=bool(int(os.environ.get("KERNEL_TRACE", "0"))))
    kernel.last_results = res
    kernel.last_meta = meta
    VM = meta["VM"]
    outs = [res.results[c]["out"].reshape(-1, VM) for c in range(n_cores)]
    return np.concatenate(outs, axis=0).astype(np.float32)


# revision 11
# speedup vs baseline: 1.9407x; 1.0978x over previous
"""Trainium2 Bass kernel: 2-layer GAT (PyG GATConv, heads=1) + per-node actor
MLP + candidate softmax, SPMD across 8 NeuronCores.

Strategy (dst-sharded data parallel):
  - Symmetrized edges (self loops handled separately), partitioned by dst
    across 8 cores, grouped into 128-dst blocks, GG blocks per gather group.
    Per (group, segment) the edges are packed block-major into one padded
    run of 128-edge chunks; a chunk may span adjacent blocks, handled by
    per-(chunk, block) matmul instances.
  - Node table per layer: bf16 [z(128) | e_src | 1.0 | pad] rows (512B).
    Per edge, dma_gather pulls the src row (segment-relative int16 idx).
  - Host precomputes the transposed one-hot (dst-on-partitions) per
    instance, streamed bf16; one matmul per instance gathers e_dst onto
    edge partitions. alpha = es + ed batched per group: DVE add + DVE
    leaky-relu + one Act exp (keeps Act inside one act-table set).
  - ohw[e,dst] = (iota==d_local)*ex built in one bf16 DVE op per instance;
    one PE matmul per instance accumulates numerator AND denominator
    (table's ones column) into the block psum. Self-loop contributions are
    added per block via a diagonal matmul from the local z-shard staging.
  - Epilogue per block: h = elu(num/den + b) (+1 trick), PE transpose,
    projection to next layer's table row + the shard-local ed table.
    Phase 0 and the layer boundary AllGather the bf16 node tables.
  - Scores are per-node scalars -> AllGather 400KB -> candidate gather +
    grouped softmax over vm=16, sharded over decisions.
"""

import math
import os
import sys

sys.path.insert(0, "/opt/trn_rl_repo")

import ml_dtypes
import numpy as np

import concourse.bass as bass
import concourse.mybir as mybir
import concourse.tile as tile
from concourse import bacc
from concourse.bass import IndirectOffsetOnAxis
from concourse.bass_utils import run_bass_kernel_spmd

F32 = mybir.dt.float32
I32 = mybir.dt.int32
I16 = mybir.dt.int16
BF16 = mybir.dt.bfloat16
ALU = mybir.AluOpType
ACTF = mybir.ActivationFunctionType
BF = ml_dtypes.bfloat16

NEG_SLOPE = 0.2
P = 128
SEGR = 32768          # table rows per int16-addressable segment
TW = 256              # bf16 table row: z(128) | es | 1.0 | pad  (512B)
GG = 3                # blocks per gather group


# ----------------------------------------------------------------- host prep
def _schedule(edge_index, N, n_cores):
    """Common chunk/instance schedule + per-core index arrays."""
    NSH = N // n_cores
    NBLK = math.ceil(NSH / P)
    NSEG = math.ceil(N / SEGR)
    e0 = edge_index[0].astype(np.int64)
    e1 = edge_index[1].astype(np.int64)
    src = np.concatenate([e0, e1])
    dst = np.concatenate([e1, e0])

    # bucket edges: per core, per block, per segment (src-sorted)
    percore = []
    for c in range(n_cores):
        m = (dst >= c * NSH) & (dst < (c + 1) * NSH)
        s_c, d_c = src[m], dst[m] - c * NSH
        o = np.lexsort((s_c, d_c // P))
        s_c, d_c = s_c[o], d_c[o]
        blk = d_c // P
        bs = np.searchsorted(blk, np.arange(NBLK), side="left")
        be = np.searchsorted(blk, np.arange(NBLK), side="right")
        per_blk = []
        for b in range(NBLK):
            sb_, db_ = s_c[bs[b]:be[b]], d_c[bs[b]:be[b]]
            seg = sb_ >> 15
            segs = []
            for s in range(NSEG):
                sm = seg == s
                segs.append((sb_[sm], db_[sm]))
            per_blk.append(segs)
        percore.append(per_blk)

    # common per (group, seg) padded run lengths (shared by all cores) and
    # per (group, seg, block) edge counts per core to derive instance spans
    ngrp = math.ceil(NBLK / GG)
    run_len = np.zeros((ngrp, NSEG), dtype=np.int64)   # padded (x128)
    for gi in range(ngrp):
        b0 = gi * GG
        g = min(GG, NBLK - b0)
        for s in range(NSEG):
            mx = 0
            for c in range(n_cores):
                tot = sum(len(percore[c][b0 + bb][s][0]) for bb in range(g))
                mx = max(mx, tot)
            run_len[gi, s] = math.ceil(mx / P) * P if mx else 0

    # groups meta: per group, per seg: chunk col base; chunk count
    groups = []          # (b0, g, segs=[(s, cb_chunk, nch)], gc)
    kk = 0
    for gi in range(ngrp):
        b0 = gi * GG
        g = min(GG, NBLK - b0)
        segs = []
        for s in range(NSEG):
            nch = int(run_len[gi, s]) // P
            segs.append((s, kk, nch))
            kk += nch
        gc = sum(x[2] for x in segs)
        groups.append((b0, g, segs))
    K = kk

    # per-core: index streams, per-instance dloc and instance schedule.
    # The instance schedule (which blocks each chunk touches) must be
    # IDENTICAL across cores (SPMD single program): merge spans over cores.
    # For each (group, seg, chunk) the set of possibly-touching blocks is
    # derived from per-core block spans; union over cores.
    inst_sets = [dict() for _ in range(ngrp)]   # (s, chunk) -> set(blocks)
    percore_edges = []
    for c in range(n_cores):
        ge = []
        for gi, (b0, g, segs) in enumerate(groups):
            for (s, cb, nch) in segs:
                if nch == 0:
                    continue
                cap = nch * P
                sp = np.zeros(cap, dtype=np.int64)
                dp = np.full(cap, 200.0, dtype=np.float32)
                bl = np.full(cap, -1, dtype=np.int64)
                off = 0
                for bb in range(g):
                    sb_, db_ = percore[c][b0 + bb][s]
                    ns = len(sb_)
                    sp[off:off + ns] = sb_ - s * SEGR
                    dp[off:off + ns] = (db_ - (b0 + bb) * P)
                    bl[off:off + ns] = bb
                    off += ns
                for k in range(nch):
                    touched = set(bl[k * P:(k + 1) * P].tolist()) - {-1}
                    key = (s, cb + k)
                    inst_sets[gi].setdefault(key, set()).update(touched)
                ge.append((gi, s, cb, nch, sp, dp, bl))
        percore_edges.append(ge)

    # canonical instance order per group: seg-major, chunk-major, block asc
    inst_meta = []       # per group: list of (s, chunk_col, bb)
    for gi, (b0, g, segs) in enumerate(groups):
        il = []
        for (s, cb, nch) in segs:
            for k in range(nch):
                bbs = sorted(inst_sets[gi].get((s, cb + k), set()))
                if not bbs:
                    bbs = [g - 1]          # dummy all-pad chunk
                for bb in bbs:
                    il.append((s, cb + k, bb))
        inst_meta.append(il)
    I = sum(len(il) for il in inst_meta)

    # build per-group kernel schedules
    gsched = []
    icol = 0
    for gi, (b0, g, segs) in enumerate(groups):
        il = inst_meta[gi]
        # ed-matmul first/last per chunk; acc last per block
        by_chunk = {}
        by_blk = {}
        insts = []
        for j, (s, cl, bb) in enumerate(il):
            by_chunk.setdefault(cl, []).append(j)
            by_blk.setdefault(bb, []).append(j)
        for j, (s, cl, bb) in enumerate(il):
            insts.append(dict(
                cl=cl, bb=bb, icol=icol + j,
                ed_first=(j == by_chunk[cl][0]),
                ed_last=(j == by_chunk[cl][-1]),
                acc_last=(j == by_blk[bb][-1])))
        gc = sum(x[2] for x in segs)
        gsched.append(dict(b0=b0, g=g, segs=segs, gc=gc, insts=insts,
                           icol0=icol, ni=len(il)))
        icol += len(il)
    assert icol == I

    meta = dict(NSH=NSH, NBLK=NBLK, NSEG=NSEG, K=K, I=I, groups=gsched)

    # per-core tensors
    out = []
    for c in range(n_cores):
        dlocI = np.full((P, I), 200.0, dtype=np.float32)
        gw_parts, goff = [], []
        go = 0
        chunk_dp = {}
        for (gi, s, cb, nch, sp, dp, bl) in percore_edges[c]:
            for k in range(nch):
                chunk_dp[(gi, s, cb + k)] = (dp[k * P:(k + 1) * P],
                                             bl[k * P:(k + 1) * P])
        for gi, (b0, g, segs) in enumerate(groups):
            slens = []
            for (gi2, s, cb, nch, sp, dp, bl) in percore_edges[c]:
                if gi2 != gi:
                    continue
                w = sp.reshape(-1, 16).T
                gw_parts.append(np.tile(w, (8, 1)).astype(np.int16))
                slens.append((s, go, len(sp)))
                go += len(sp) // 16
            goff.append(slens)
            for inst in gsched[gi]["insts"]:
                cl, bb, ic = inst["cl"], inst["bb"], inst["icol"]
                # find seg of this chunk
                for (s, cb, nch) in segs:
                    if cb <= cl < cb + nch:
                        break
                dpk, blk_ = chunk_dp.get((gi, s, cl), (None, None))
                if dpk is None:
                    continue
                v = np.where(blk_ == bb, dpk, 200.0)
                dlocI[:, ic] = v
        ohT = (np.arange(P, dtype=np.float32)[:, None, None]
               == dlocI.T[None, :, :])
        out.append(dict(
            gidx16=np.concatenate(gw_parts, axis=1),
            dloc=dlocI.astype(BF),
            ohT=np.ascontiguousarray(ohT.reshape(P, I * P)).astype(BF)))
    meta["goff"] = goff
    return meta, out


def _prep_inputs(inputs, n_cores=8):
    N, IN_DIM = inputs["state_wf"].shape
    HID = inputs["W0"].shape[1]
    VM = 16
    B = inputs["candidate_task_index"].shape[0] // VM
    meta, per_core_e = _schedule(inputs["edge_index"], N, n_cores)
    meta.update(N=N, IN_DIM=IN_DIM, HID=HID, VM=VM, B=B,
                NPAD=math.ceil(N / P) * P)

    f = lambda x: np.asarray(x, dtype=np.float32)
    W0, W1 = f(inputs["W0"]), f(inputs["W1"])
    w0big = np.concatenate(
        [W0, (W0 @ f(inputs["a_src0"]))[:, None],
         (W0 @ f(inputs["a_dst0"]))[:, None]], axis=1)
    w1big = np.concatenate(
        [W1, (W1 @ f(inputs["a_src1"]))[:, None],
         (W1 @ f(inputs["a_dst1"]))[:, None]], axis=1)
    swt = np.zeros((IN_DIM, meta["NPAD"]), dtype=np.float32)
    swt[:, :N] = f(inputs["state_wf"]).T
    NSH, NBLK = meta["NSH"], meta["NBLK"]
    common = dict(
        w0big=w0big.astype(np.float32),
        w1big=w1big.astype(np.float32),
        b0t=np.tile(f(inputs["b0"])[None, :], (P, 1)).astype(np.float32),
        b1t=np.tile(f(inputs["b1"])[None, :], (P, 1)).astype(np.float32),
        mw0=f(inputs["mW0"]),
        mw1=f(inputs["mW1"]).reshape(HID, 1),
        mb0=f(inputs["mb0"]).reshape(HID, 1),
        iota=np.tile(np.arange(P, dtype=np.float32)[None, :],
                     (P, 1)).astype(BF),
        iotac=np.arange(P, dtype=np.float32).reshape(P, 1).astype(BF),
        ident=np.eye(P, dtype=np.float32),
    )
    cand = inputs["candidate_task_index"].astype(np.int64)
    CPC = (B // n_cores) * VM
    CC = CPC // P
    meta["CC"] = CC
    in_maps = []
    for c in range(n_cores):
        m = dict(common)
        m.update(per_core_e[c])
        m["swtsh"] = np.ascontiguousarray(
            np.pad(swt[:, c * NSH:(c + 1) * NSH],
                   ((0, 0), (0, NBLK * P - NSH))))
        m["cidx"] = cand[c * CPC:(c + 1) * CPC].reshape(P, CC).astype(np.int32)
        in_maps.append(m)
    return meta, in_maps


# ------------------------------------------------------------------ builder
def build(meta, n_cores=8):
    NSH, NBLK, NSEG = meta["NSH"], meta["NBLK"], meta["NSEG"]
    K, I, groups = meta["K"], meta["I"], meta["groups"]
    N, IN_DIM, HID = meta["N"], meta["IN_DIM"], meta["HID"]
    NPAD, CC, VM = meta["NPAD"], meta["CC"], meta["VM"]
    goff = meta["goff"]
    NTROW = NSEG * SEGR
    IWG = sum(ln // 16 for slens in goff for (_, _, ln) in slens)
    max_gc = max(gs["gc"] for gs in groups)
    max_ni = max(gs["ni"] for gs in groups)

    SP = bool(int(os.environ.get("KERNEL_SP", "0")))
    nc = bacc.Bacc("TRN2", target_bir_lowering=False, debug=False,
                   enable_asserts=False, num_devices=n_cores)

    inp = {}
    for name, shape, dt in [
        ("swtsh", [IN_DIM, NBLK * P], F32),
        ("w0big", [IN_DIM, HID + 2], F32), ("w1big", [HID, HID + 2], F32),
        ("b0t", [P, HID], F32), ("b1t", [P, HID], F32),
        ("mw0", [HID, HID], F32), ("mw1", [HID, 1], F32),
        ("mb0", [HID, 1], F32), ("iota", [P, P], BF16),
        ("iotac", [P, 1], BF16), ("ident", [P, P], F32),
        ("dloc", [P, I], BF16), ("ohT", [P, I * P], BF16),
        ("gidx16", [P, IWG], I16), ("cidx", [P, CC], I32),
    ]:
        inp[name] = nc.dram_tensor(name, shape, dt, kind="ExternalInput")
    out_t = nc.dram_tensor("out", [P, CC], F32, kind="ExternalOutput")

    z0_shard = nc.dram_tensor("z0shard", [NBLK * P, TW], BF16, kind="Internal")
    z0_tab = nc.dram_tensor("z0tab", [NTROW, TW], BF16, kind="Internal",
                            addr_space="Shared")
    z1_tab = nc.dram_tensor("z1tab", [NTROW, TW], BF16, kind="Internal",
                            addr_space="Shared")
    z1_shard = nc.dram_tensor("z1shard", [NBLK * P, TW], BF16, kind="Internal")
    ed_tab = [nc.dram_tensor(f"ed{l}tab", [NBLK * P, 1], BF16, kind="Internal")
              for l in range(2)]
    sc_shard = nc.dram_tensor("scshard", [NBLK, P, 1], F32, kind="Internal")
    sc_full = nc.dram_tensor("scfull", [N, 1], F32, kind="Internal",
                             addr_space="Shared")

    with tile.TileContext(nc) as tc:
        with (
            tc.tile_pool(name="const", bufs=1) as cpool,
            tc.tile_pool(name="stream", bufs=2) as spool,
            tc.tile_pool(name="idxs", bufs=5) as ipool,
            tc.tile_pool(name="zrows", bufs=3) as zpool,
            tc.tile_pool(name="ohts", bufs=3) as opool,
            tc.tile_pool(name="zown", bufs=6) as znpool,
            tc.tile_pool(name="work", bufs=3) as wpool,
            tc.tile_pool(name="ohwp", bufs=6) as ohwpool,
            tc.tile_pool(name="stage", bufs=3) as stpool,
            tc.tile_pool(name="psacc", bufs=3, space="PSUM") as psacc,
            tc.tile_pool(name="psaps", bufs=2, space="PSUM") as psaps,
            tc.tile_pool(name="pstp", bufs=1, space="PSUM") as pstp,
            tc.tile_pool(name="psproj", bufs=1, space="PSUM") as psproj,
            tc.tile_pool(name="pssc", bufs=1, space="PSUM") as pssc,
        ):
            sb = {}
            for name in ("w0big", "w1big", "b0t", "b1t", "mw0", "mw1", "mb0",
                         "iota", "iotac", "ident", "dloc", "cidx"):
                t = inp[name]
                dt = {"cidx": I32, "iota": BF16, "iotac": BF16,
                      "dloc": BF16}.get(name, F32)
                sb[name] = cpool.tile(list(t.shape), dt, tag=name, name=name)
                nc.sync.dma_start(sb[name][:], t[:])

            # ===== phase 0: shard z0 projection + ed0 table + AllGather ====
            NSTG = 8
            for sb0 in range(0, NBLK, 16):
                sbn = min(16, NBLK - sb0)
                swsh_sb = spool.tile([IN_DIM, 16 * P], F32, tag="swt",
                                     name="swsh")
                nc.sync.dma_start(swsh_sb[:, :sbn * P],
                                  inp["swtsh"][:, sb0 * P:(sb0 + sbn) * P])
                for j0 in range(0, sbn, NSTG):
                    jn = min(NSTG, sbn - j0)
                    stg = stpool.tile([P, NSTG * TW], BF16, tag="z0st",
                                      name="z0stg")
                    nc.vector.memset(stg[:], 1.0)
                    estg = stpool.tile([P, NSTG], BF16, tag="edst",
                                       name="edstg")
                    for j in range(jn):
                        bl = j0 + j
                        ps = psacc.tile([P, HID + 2], F32, tag="acc",
                                        name="z0ps")
                        nc.tensor.matmul(
                            ps[:], swsh_sb[:, bl * P:(bl + 1) * P],
                            sb["w0big"][:], start=True, stop=True)
                        if j % 2 == 0:
                            nc.scalar.copy(stg[:, j * TW:j * TW + HID + 1],
                                           ps[:, :HID + 1])
                        else:
                            nc.vector.tensor_copy(
                                stg[:, j * TW:j * TW + HID + 1],
                                ps[:, :HID + 1])
                        nc.vector.tensor_copy(estg[:, j:j + 1],
                                              ps[:, HID + 1:HID + 2])
                    r0 = (sb0 + j0) * P
                    nc.sync.dma_start(
                        z0_shard[r0:r0 + jn * P].rearrange("(j p) c -> p j c",
                                                           p=P),
                        stg[:, :jn * TW].rearrange("p (j c) -> p j c", c=TW))
                    nc.sync.dma_start(
                        ed_tab[0][r0:r0 + jn * P].rearrange(
                            "(j p) c -> p j c", p=P),
                        estg[:, :jn].rearrange("p (j c) -> p j c", c=1))
            nc.gpsimd.collective_compute(
                "AllGather", ALU.bypass,
                replica_groups=[list(range(n_cores))],
                ins=[z0_shard[:].flatten()[0:NSH * TW].opt()],
                outs=[z0_tab[0:N].flatten().opt()])

            # ======================= GAT layers ============================
            for layer in range(2):
                btile = sb["b0t"] if layer == 0 else sb["b1t"]
                tab = z0_tab if layer == 0 else z1_tab
                shard = z0_shard if layer == 0 else z1_shard
                for gi_, gs in enumerate(groups):
                    gb0, g, gc, ni = gs["b0"], gs["g"], gs["gc"], gs["ni"]
                    ic0 = gs["icol0"]
                    zr = zpool.tile([P, max_gc, TW], BF16, tag="zr", name="zr")
                    oht = opool.tile([P, max_ni * P], BF16, tag="oht",
                                     name="oht")
                    nc.sync.dma_start(oht[:, :ni * P],
                                      inp["ohT"][:, ic0 * P:(ic0 + ni) * P])
                    edc = ipool.tile([P, GG], BF16, tag="edc", name="edc")
                    nc.sync.dma_start(
                        edc[:, :g],
                        ed_tab[layer][gb0 * P:(gb0 + g) * P].rearrange(
                            "(j p) c -> p (j c)", p=P))
                    # --- gathers (per index segment) ---
                    for (s, goff_s, ln) in goff[gi_]:
                        gidx = ipool.tile([P, max(ln // 16, 1)], I16,
                                          tag="gidx", name="gidxt")
                        nc.sync.dma_start(
                            gidx[:, :ln // 16],
                            inp["gidx16"][:, goff_s:goff_s + ln // 16])
                        for (s_, cb, nch) in gs["segs"]:
                            if s_ == s:
                                break
                        assert s_ == s and nch == ln // P
                        nc.gpsimd.dma_gather(
                            out_ap=zr[:, cb - gs["segs"][0][1]:
                                      cb - gs["segs"][0][1] + nch, :],
                            in_ap=tab[s * SEGR:(s + 1) * SEGR],
                            idxs_ap=gidx[:, :ln // 16],
                            num_idxs=ln, num_idxs_reg=ln, elem_size=TW,
                            single_packet=SP)
                    # --- self-loop diagonal per block (opens psum accum) ---
                    bps = {}
                    for bi in range(g):
                        b = gb0 + bi
                        zo = znpool.tile([P, TW], BF16, tag="zo", name="zo")
                        nc.sync.dma_start(zo[:], shard[b * P:(b + 1) * P])
                        sxa = wpool.tile([P, 1], F32, tag="sxa", name="sxa")
                        nc.vector.tensor_tensor(
                            out=sxa[:], in0=zo[:, HID:HID + 1],
                            in1=edc[:, bi:bi + 1], op=ALU.add)
                        sxl = wpool.tile([P, 1], F32, tag="sxl", name="sxl")
                        nc.vector.scalar_tensor_tensor(
                            out=sxl[:], in0=sxa[:], scalar=NEG_SLOPE,
                            in1=sxa[:], op0=ALU.mult, op1=ALU.max)
                        sx = wpool.tile([P, 1], BF16, tag="sx", name="sx")
                        nc.scalar.activation(out=sx[:], in_=sxl[:],
                                             func=ACTF.Exp)
                        dg = ohwpool.tile([P, P], BF16, tag="ohw", name="dg")
                        nc.vector.scalar_tensor_tensor(
                            out=dg[:], in0=sb["iota"][:],
                            scalar=sb["iotac"][:],
                            in1=sx[:].to_broadcast([P, P]),
                            op0=ALU.is_equal, op1=ALU.mult)
                        bps[bi] = psacc.tile([P, HID + 2], F32, tag="acc",
                                             name="bps")
                        nc.tensor.matmul(
                            bps[bi][:], dg[:], zo[:, 0:HID + 2],
                            start=True, stop=(len([i for i in gs["insts"]
                                                   if i["bb"] == bi]) == 0),
                            skip_group_check=True)
                    # --- pass 1: ed per edge via one-hot matmuls ---
                    aps = psaps.tile([P, max_gc], F32, tag="aps", name="aps")
                    for inst in gs["insts"]:
                        cl = inst["cl"] - gs["segs"][0][1]
                        lc = inst["icol"] - ic0
                        nc.tensor.matmul(
                            aps[:, cl:cl + 1],
                            oht[:, lc * P:(lc + 1) * P],
                            edc[:, inst["bb"]:inst["bb"] + 1],
                            start=inst["ed_first"], stop=inst["ed_last"],
                            skip_group_check=True)
                    # --- alpha = es + ed, lrelu, exp (batched per group) ---
                    tse = wpool.tile([P, max_gc], F32, tag="tse", name="tse")
                    nc.vector.tensor_tensor(
                        out=tse[:, :gc], in0=aps[:, :gc],
                        in1=zr[:, :gc, HID:HID + 1].rearrange(
                            "p a b -> p (a b)"),
                        op=ALU.add)
                    lr = wpool.tile([P, max_gc], F32, tag="lr", name="lr")
                    nc.vector.scalar_tensor_tensor(
                        out=lr[:, :gc], in0=tse[:, :gc], scalar=NEG_SLOPE,
                        in1=tse[:, :gc], op0=ALU.mult, op1=ALU.max)
                    exc = wpool.tile([P, max_gc], BF16, tag="exc", name="exc")
                    nc.scalar.activation(out=exc[:, :gc], in_=lr[:, :gc],
                                         func=ACTF.Exp)
                    # --- pass 2: weighted one-hot + scatter matmuls ---
                    if layer == 0:
                        z1stg = stpool.tile([P, GG * TW], BF16, tag="z1st",
                                            name="z1stg")
                        nc.vector.memset(z1stg[:], 1.0)
                        ed1stg = stpool.tile([P, GG], BF16, tag="ed1st",
                                             name="ed1stg")
                    else:
                        scstg = stpool.tile([1, GG * P], F32, tag="scst",
                                            name="scstg")
                    for inst in gs["insts"]:
                        cl = inst["cl"] - gs["segs"][0][1]
                        ohw = ohwpool.tile([P, P], BF16, tag="ohw",
                                           name="ohw")
                        nc.vector.scalar_tensor_tensor(
                            out=ohw[:], in0=sb["iota"][:],
                            scalar=sb["dloc"][:, inst["icol"]:
                                              inst["icol"] + 1],
                            in1=exc[:, cl:cl + 1].to_broadcast([P, P]),
                            op0=ALU.is_equal, op1=ALU.mult)
                        nc.tensor.matmul(
                            bps[inst["bb"]][:], ohw[:],
                            zr[:, cl:cl + 1, 0:HID + 2].squeeze(),
                            start=False, stop=inst["acc_last"],
                            skip_group_check=True)
                    # --- epilogues (all chunks of the group are done) ---
                    for bi in range(g):
                        b = gb0 + bi
                        pb = bps[bi]
                        rc = wpool.tile([P, 1], F32, tag="rc", name="rc")
                        nc.vector.reciprocal(rc[:], pb[:, HID + 1:HID + 2])
                        y = wpool.tile([P, HID], F32, tag="y", name="y")
                        nc.vector.scalar_tensor_tensor(
                            out=y[:], in0=pb[:, :HID], scalar=rc[:],
                            in1=btile[:], op0=ALU.mult, op1=ALU.add)
                        e_t = wpool.tile([P, HID], F32, tag="e_t", name="e_t")
                        r_t = wpool.tile([P, HID], F32, tag="r_t", name="r_t")
                        nc.scalar.activation(out=e_t[:], in_=y[:],
                                             func=ACTF.Exp)
                        nc.scalar.activation(out=r_t[:], in_=y[:],
                                             func=ACTF.Relu)
                        hp1 = wpool.tile([P, HID], F32, tag="hp1", name="hp1")
                        nc.vector.scalar_tensor_tensor(
                            out=hp1[:], in0=e_t[:], scalar=1.0, in1=r_t[:],
                            op0=ALU.min, op1=ALU.add)      # elu(y) + 1
                        tp = pstp.tile([P, P], F32, tag="tp", name="tp")
                        nc.tensor.transpose(tp[:], hp1[:], sb["ident"][:])
                        hT = wpool.tile([P, HID], F32, tag="hT", name="hT")
                        nc.scalar.activation(out=hT[:], in_=tp[:],
                                             func=ACTF.Copy, bias=-1.0)
                        if layer == 0:
                            zps = psproj.tile([P, HID + 2], F32, tag="proj",
                                              name="zps")
                            nc.tensor.matmul(zps[:], hT[:], sb["w1big"][:],
                                             start=True, stop=True,
                                             skip_group_check=True)
                            nc.scalar.copy(
                                z1stg[:, bi * TW:bi * TW + HID + 1],
                                zps[:, :HID + 1])
                            nc.vector.tensor_copy(
                                ed1stg[:, bi:bi + 1],
                                zps[:, HID + 1:HID + 2])
                        else:
                            mps = psproj.tile([P, HID], F32, tag="proj",
                                              name="mps")
                            nc.tensor.matmul(mps[:], sb["mw0"][:], hT[:],
                                             start=True, stop=True,
                                             skip_group_check=True)
                            m1 = wpool.tile([P, HID], F32, tag="m1", name="m1")
                            nc.scalar.activation(out=m1[:], in_=mps[:],
                                                 func=ACTF.Relu,
                                                 bias=sb["mb0"][:])
                            sps = pssc.tile([1, P], F32, tag="sc",
                                            name="sps")
                            nc.tensor.matmul(sps[:], sb["mw1"][:], m1[:],
                                             start=True, stop=True,
                                             skip_group_check=True)
                            nc.scalar.copy(scstg[:, bi * P:(bi + 1) * P],
                                           sps[:])
                    if layer == 0:
                        nc.sync.dma_start(
                            z1_shard[gb0 * P:(gb0 + g) * P].rearrange(
                                "(j p) c -> p j c", p=P),
                            z1stg[:, :g * TW].rearrange("p (j c) -> p j c",
                                                        c=TW))
                        nc.sync.dma_start(
                            ed_tab[1][gb0 * P:(gb0 + g) * P].rearrange(
                                "(j p) c -> p j c", p=P),
                            ed1stg[:, :g].rearrange("p (j c) -> p j c", c=1))
                    else:
                        nc.sync.dma_start(sc_shard[gb0:gb0 + g],
                                          scstg[:, :g * P])
                if layer == 0:
                    nc.gpsimd.collective_compute(
                        "AllGather", ALU.bypass,
                        replica_groups=[list(range(n_cores))],
                        ins=[z1_shard[:].flatten()[0:NSH * TW].opt()],
                        outs=[z1_tab[0:N].flatten().opt()])

            # ================= scores + candidate softmax ==================
            nc.gpsimd.collective_compute(
                "AllGather", ALU.bypass,
                replica_groups=[list(range(n_cores))],
                ins=[sc_shard[:].flatten()[0:NSH].opt()],
                outs=[sc_full[:].flatten().opt()])
            scg = wpool.tile([P, CC], F32, tag="scg", name="scg")
            for c in range(CC):
                nc.gpsimd.indirect_dma_start(
                    out=scg[:, c:c + 1], out_offset=None, in_=sc_full[:],
                    in_offset=IndirectOffsetOnAxis(
                        ap=sb["cidx"][:, c:c + 1], axis=0))
            NG = CC // VM
            pex = wpool.tile([P, CC], F32, tag="pex", name="pex")
            nc.scalar.activation(out=pex[:], in_=scg[:], func=ACTF.Exp)
            ssum = wpool.tile([P, NG], F32, tag="ssum", name="ssum")
            nc.vector.tensor_reduce(
                out=ssum[:], in_=pex[:].rearrange("p (g v) -> p g v", v=VM),
                axis=mybir.AxisListType.X, op=ALU.add)
            rcg = wpool.tile([P, NG], F32, tag="rcg", name="rcg")
            nc.vector.reciprocal(rcg[:], ssum[:])
            pi = wpool.tile([P, CC], F32, tag="pi", name="pi")
            for g_ in range(NG):
                nc.vector.tensor_scalar(
                    out=pi[:, g_ * VM:(g_ + 1) * VM],
                    in0=pex[:, g_ * VM:(g_ + 1) * VM],
                    scalar1=rcg[:, g_:g_ + 1], scalar2=0.0,
                    op0=ALU.mult, op1=ALU.add)
            nc.sync.dma_start(out_t[:], pi[:])

    return nc


# ------------------------------------------------------------------- kernel
def kernel(**inputs):
    n_cores = 8
    meta, in_maps = _prep_inputs(inputs, n_cores)
    nc = build(meta, n_cores)
    nc.compile()
    res = run_bass_kernel_spmd(
        nc, in_maps, core_ids=list(range(n_cores)),
        trace=bool(int(os.environ.get("KERNEL_TRACE", "0"))))
    kernel.last_results = res
    kernel.last_meta = meta
    VM = meta["VM"]
    outs = [res.results[c]["out"].reshape(-1, VM) for c in range(n_cores)]
    return np.concatenate(outs, axis=0).astype(np.float32)
